# revision 37
# baseline (speedup 1.0000x reference)
"""Multi-head attention (B=2, S=2048, D=1024, H=16, causal + key-pad mask)
as an 8-core Trainium2 Bass/Tile SPMD kernel.

Sharding: data parallel over the 2 batches (4 cores each); within a batch
group, tensor parallel over heads (4 heads/core) for the QKV projections and
attention. The O-projection is ROW-parallel: each core multiplies its 4
normalized head outputs by its 256 rows of Wo, producing a full-width
[S, 1024] partial sum; the host adds the 4 partials per batch (plus bo).
No device collectives at all.

Key compaction: the pad mask kills ~half the keys, and masked keys contribute
exactly 0 to softmax (exp(-1e9/8) underflows) in the reference too. The host
compacts K/V work to the unmasked key positions (padded to a multiple of 128,
exp-bias NEG on the padding), roughly halving the QK/AV matmuls, the exp
work, and the K/V projections. Causal masking in compacted key space is
data-dependent, so the host precomputes NEG/0 mask tiles for the few key
blocks that straddle each q-tile's causal boundary; fully-past blocks are
never emitted, fully-valid blocks need no mask.

All matmul operands are bf16 (fp32 accumulation in PSUM). Softmax skips
max-subtraction (scores are O(5) here), applies the key-pad mask through the
exp bias and the causal boundary masks via DVE adds. Softmax denominators
ride along as a ones-column in the V operand; normalization uses the fast
approximate DVE reciprocal and an f32r PE ones-broadcast.

Startup: the first projection's inputs are DMA'd first; bulk loads are
triggered from the vector engine's queue after the first projection's bias
add, so they cannot steal HBM bandwidth from the critical path.

self-contained: includes a workaround for the walrus per-instruction
sync-wait limit and an NTFF-profile hook shim.
"""
import sys
import types

import numpy as np

import bass_rust
import concourse.bass as bass
import concourse.mybir as mybir
import concourse.tile as tile


# ---- walrus sync-wait limit workaround ----------------------------------
# This walrus build rejects instructions carrying more than one sem wait
# ("Too many sync wait commands"). Tile emits multi-wait instructions (the
# final drain, matmuls waiting on several DMA queues). Split excess waits
# onto same-engine NoOps placed immediately before the instruction --
# serial waits on one sequencer are semantically identical.
_WSPLIT_COUNTER = [0]


def _split_excess_waits(nc, limit=1):
    for fn in nc.m.functions:
        for bb in fn.blocks:
            out = []
            changed = False
            for inst in bb.instructions:
                si = inst.sync_info
                waits = list(si.on_wait) if si is not None and si.on_wait else []
                if len(waits) > limit:
                    extra, keep = waits[:-limit], waits[-limit:]
                    for s in range(0, len(extra), limit):
                        _WSPLIT_COUNTER[0] += 1
                        nop = mybir.InstNoOp(
                            name=f"I-wsplit-{_WSPLIT_COUNTER[0]}", ins=[], outs=[]
                        )
                        nop.engine = inst.engine
                        nop.sync_info = bass_rust.SyncInfo(
                            on_wait=extra[s : s + limit], on_update=[]
                        )
                        out.append(nop)
                    si.on_wait = keep
                    changed = True
                out.append(inst)
            if changed:
                bb.instructions = out


def _install_tile_patch():
    if getattr(tile.TileContext, "_wait_split_patched", False):
        return
    orig_exit = tile.TileContext.__exit__

    def __exit__(self, exc_type, exc_val, exc_tb):
        r = orig_exit(self, exc_type, exc_val, exc_tb)
        if exc_type is None:
            _split_excess_waits(self.nc)
        return r

    tile.TileContext.__exit__ = __exit__
    tile.TileContext._wait_split_patched = True


_install_tile_patch()


# ---- NTFF profile hook shim (axon deployments missing antenv.axon_hooks) --
def _install_ntff_hook():
    try:
        import antenv.axon_hooks  # noqa: F401
        return
    except ImportError:
        pass
    try:
        from trn_agent_boot.trn_boot import _ntff_profile_via_ctypes

        hook = _ntff_profile_via_ctypes("/opt/axon/libaxon_pjrt.so")
    except Exception:
        hook = None
    m = types.ModuleType("antenv.axon_hooks")
    m.get_axon_ntff_profile_hook = lambda: hook
    m.set_axon_ntff_profile_hook = lambda h: None
    sys.modules["antenv.axon_hooks"] = m


_install_ntff_hook()

import concourse.bass_utils as bass_utils  # noqa: E402
from concourse.bass_utils import run_bass_kernel_spmd  # noqa: E402


# note: --enable-ldw-opt=true was tried here and crashes this walrus build's
# codegen (visitInstLdweights, CoreV3GenImpl.cpp:694) — it is off for a
# reason; LDWEIGHTS overlap must come from instruction scheduling instead.

f32 = mybir.dt.float32
f32r = mybir.dt.float32r
bf16 = mybir.dt.bfloat16

B, S, D, H, HD = 2, 2048, 1024, 16, 64
HPC, GROUP = 4, 4          # heads per core, cores per batch
HC = HPC * HD              # 256 projection cols per core
NQT = S // 512             # 4 q-tiles
QT = 512                   # q-tile width
NJT = D // 128             # 8 output column tiles (full width, row-parallel)
SCALE = 1.0 / np.sqrt(HD)  # 0.125
NEG = -1.0e9
KCH = D // 128             # 8 contraction chunks


class Cfg:
    """Compile-time attention geometry derived from the runtime pad_mask."""

    def __init__(self, pad_mask):
        pad_mask = np.asarray(pad_mask)
        self.keys = [np.flatnonzero(~pad_mask[b]) for b in range(B)]
        self.scb = [len(k) for k in self.keys]
        self.nktc = -(-max(self.scb) // 128)
        self.sc = self.nktc * 128
        cnt = [
            [int((self.keys[b] < (qi + 1) * QT).sum()) for qi in range(NQT)]
            for b in range(B)
        ]
        self.nktq = [
            max(-(-cnt[b][qi] // 128) for b in range(B)) for qi in range(NQT)
        ]
        minpos, maxpos = [], []
        for kt in range(self.nktc):
            mn, mx = S, -1
            for b in range(B):
                lo, hi = kt * 128, min(kt * 128 + 128, self.scb[b])
                if lo < hi:
                    mn = min(mn, int(self.keys[b][lo]))
                    mx = max(mx, int(self.keys[b][hi - 1]))
            minpos.append(mn)
            maxpos.append(mx)
        self.d0 = [
            [max(0, minpos[kt] - qi * QT) for kt in range(self.nktq[qi])]
            for qi in range(NQT)
        ]
        self.mask_order = []          # [(qi, kt)]
        self.mask_idx = {}
        for qi in range(NQT):
            for kt in range(self.nktq[qi]):
                if maxpos[kt] > qi * QT:
                    self.mask_idx[(qi, kt)] = len(self.mask_order)
                    self.mask_order.append((qi, kt))
        self.nmask = len(self.mask_order)
        self.nct = -(-self.sc // 512)  # k-proj column tiles
        # masks needed before attention(qi) starts: index of first mask of qi>0
        self.nmask_q0 = sum(1 for (qi, _) in self.mask_order if qi == 0)
        self.key = (
            self.sc,
            tuple(self.nktq),
            tuple(tuple(r) for r in self.d0),
            tuple(self.mask_order),
        )


def build(cfg):
    nc = bass.Bass()
    dp = nc.declare_dram_parameter
    # xS[c, p, k, j] = x[c*512+j, k*128+p]: contiguous 8KiB per partition.
    xS = dp("xS", [NQT, 128, KCH, QT], bf16, isOutput=False)
    # xcA/xcB[p, k, j] = x[keys[j'], k*128+p] over compacted keys.
    xcA = dp("xcA", [128, KCH, min(cfg.sc, 512)], bf16, isOutput=False)
    if cfg.sc > 512:
        xcB = dp("xcB", [128, KCH, cfg.sc - 512], bf16, isOutput=False)
    else:
        xcB = None
    # w*f[p, jt, k, j] = W.T[k*128+p, jt*128+j] over this core's 256 cols.
    wqf = dp("wqf", [128, 2, KCH, 128], bf16, isOutput=False)
    wkf = dp("wkf", [128, 2, KCH, 128], bf16, isOutput=False)
    wvf = dp("wvf", [128, KCH, HC], bf16, isOutput=False)
    # wof[p, pair, jt, j]: Wo rows for this core's heads, pair-chunked.
    wof = dp("wof", [128, 2, NJT, 128], bf16, isOutput=False)
    bq = dp("bq", [128, 2], f32, isOutput=False)
    bk = dp("bk", [128, 2], f32, isOutput=False)
    bv = dp("bv", [1, HC], bf16, isOutput=False)
    padb = dp("padb", [128, cfg.nktc], f32, isOutput=False)
    if cfg.nmask:
        cmask = dp("cmask", [128, cfg.nmask, QT], bf16, isOutput=False)
    else:
        cmask = None
    out = dp("out", [NQT, NJT, 128, QT], bf16, isOutput=True)

    with tile.TileContext(nc) as tc:
        _body(nc, tc, cfg, locals())
    # populate .instr bytes for extended-inst ISA ops (custom DVE reciprocal);
    # Bacc.compile runs this pass but the raw-Bass path does not.
    from concourse.library_overlay import lower_extended_insts

    lower_extended_insts(nc)
    return nc


def _body(nc, tc, cfg, t):
    xS, xcA, xcB, wqf, wkf, wvf, wof = (
        t["xS"], t["xcA"], t["xcB"], t["wqf"], t["wkf"], t["wvf"], t["wof"]
    )
    bq, bk, bv, padb, cmask, out = (
        t["bq"], t["bk"], t["bv"], t["padb"], t["cmask"], t["out"]
    )
    SC, NKTC, NCT = cfg.sc, cfg.nktc, cfg.nct

    ctx_pools = []

    def pool(name, bufs, space="SBUF"):
        p = tc.tile_pool(name=name, bufs=bufs, space=space)
        ctx_pools.append(p)
        return p.__enter__()

    const = pool("const", 1)
    probs_pool = pool("probs", 4)
    stage_pool = pool("stage", 2)
    heads_pool = pool("heads", 4)
    outsb_pool = pool("outsb", 3)

    ps_proj = pool("ps_proj", 2, space="PSUM")
    ps_st = pool("ps_st", 2, space="PSUM")
    ps_ot = pool("ps_ot", 2, space="PSUM")

    ones_f = const.tile([128, 128], f32)
    nc.any.memset(ones_f[:], 1.0)
    ones_b = const.tile([1, 128], bf16)
    with nc.allow_low_precision(reason="exact small ints in bf16"):
        nc.vector.tensor_copy(ones_b[0:1, :], ones_f[0:1, :])

    # HAM warmup: the PE would otherwise idle ~15us waiting for the input
    # DMAs and the first ~3.4us of real matmuls would run at the 1.2 GHz
    # cold clock. Burn the wait on dummy matmuls (results never read) so the
    # activity monitor reaches 8/8 before the first projection.
    zdum = const.tile([128, QT], bf16)
    nc.vector.memset(zdum[:], 0.0)
    for i in range(20):
        psd = ps_st.tile([128, QT], f32, tag="st", name=f"psd{i}")
        nc.tensor.matmul(psd[:], zdum[:, 0:128], zdum[:], start=True, stop=True)
    # preload the ACT exp table during the input-DMA wait — otherwise the
    # 1.3us ACT_TABLE_LOAD serializes in front of the first real exp
    warm_e = const.tile([1, 2], f32)
    nc.scalar.activation(
        warm_e[:], ones_f[0:1, 0:2], mybir.ActivationFunctionType.Exp,
        scale=1.0,
    )

    # ---- small consts first (tiny DMAs) ---------------------------------
    bq_t = const.tile([128, 2], f32)
    nc.sync.dma_start(bq_t[:], bq[:])
    bk_t = const.tile([128, 2], f32)
    nc.sync.dma_start(bk_t[:], bk[:])
    padb_t = const.tile([128, NKTC], f32)
    nc.sync.dma_start(padb_t[:], padb[:])
    bv_row = const.tile([1, HC], bf16)
    nc.sync.dma_start(bv_row[:], bv[:])

    # ---- critical-path inputs in dependency order ------------------------
    xt = const.tile([128, NQT, KCH, QT], bf16)   # full x, s-slice-major
    xc_t = const.tile([128, KCH, SC], bf16)      # compacted-key x
    wq_t = const.tile([128, 2, KCH, 128], bf16)
    wk_t = const.tile([128, 2, KCH, 128], bf16)
    wv_t = const.tile([128, KCH, HC], bf16)
    wo_t = const.tile([128, 2, NJT, 128], bf16)
    if cfg.nmask:
        cm_t = const.tile([128, cfg.nmask, QT], bf16)
    else:
        cm_t = None

    # Each dma_start occupies ONE of the 16 DMA rings (~20-25 GB/s each);
    # aggregate HBM bandwidth needs ~16 concurrent transfers, and rings are
    # assigned round-robin in emission order and drain FIFO. So: chunk every
    # startup tensor ~16 ways — emission order then IS the priority order at
    # full bandwidth, with no artificial gating needed between waves.
    CA = min(SC, 512)
    # wave 1: k-projection inputs (wk both jt + xcA), 12 chunks
    for k4 in range(0, KCH, 4):
        nc.sync.dma_start(wk_t[:, 0, k4 : k4 + 4], wkf[:, 0, k4 : k4 + 4])
    for k4 in range(0, KCH, 4):
        nc.sync.dma_start(wk_t[:, 1, k4 : k4 + 4], wkf[:, 1, k4 : k4 + 4])
    for k in range(KCH):
        nc.sync.dma_start(xc_t[:, k, 0:CA], xcA[:, k, :])
    # wave 2: v-projection (small, first) then q-projection inputs
    for k4 in range(0, KCH, 4):
        nc.sync.dma_start(wv_t[:, k4 : k4 + 4], wvf[:, k4 : k4 + 4])
    for k4 in range(0, KCH, 4):
        nc.sync.dma_start(wq_t[:, 0, k4 : k4 + 4], wqf[:, 0, k4 : k4 + 4])
    for k4 in range(0, KCH, 4):
        nc.sync.dma_start(wq_t[:, 1, k4 : k4 + 4], wqf[:, 1, k4 : k4 + 4])
    for k in range(KCH):
        nc.sync.dma_start(xt[:, 0, k, :], xS[0, :, k, :])
    if cfg.nmask and cfg.nmask_q0:
        nc.sync.dma_start(cm_t[:, 0 : cfg.nmask_q0], cmask[:, 0 : cfg.nmask_q0])

    bvb = const.tile([128, HC], f32)
    bv_ps = ps_proj.tile([128, HC], f32, tag="proj")
    nc.tensor.matmul(bv_ps[:], ones_b[:], bv_row[:], start=True, stop=True)
    nc.vector.tensor_copy(bvb[:], bv_ps[:])

    # projection outputs
    qh_t = const.tile([128, 2, S], bf16)    # qhT: [j-in-tile, j-tile, s]
    kh_t = const.tile([128, 2, SC], bf16)
    vh_t = const.tile([128, NKTC, HPC, HD + 1], bf16)
    with nc.allow_low_precision(reason="exact small ints in bf16"):
        nc.vector.tensor_copy(
            vh_t[:, :, :, HD : HD + 1].rearrange("p t h o -> p (t h) o"),
            ones_f[:, 0 : NKTC * HPC].rearrange("p (f o) -> p f o", o=1),
        )

    def bulk_loads():
        """Deferred loads. The dummy copy makes the first bulk DMA depend on
        the last critical load (xS[0]); since the SP queue dispatches DMA
        triggers in program order, every bulk transfer queues behind it and
        cannot starve the critical-path loads of HBM bandwidth."""
        with nc.allow_low_precision(reason="dummy gate, overwritten"):
            nc.vector.tensor_copy(
                xt[:, 1, 0, 0:1], xt[:, 0, KCH - 1, QT - 1 : QT]
            )
        for k in range(KCH):
            nc.sync.dma_start(xt[:, 1, k, :], xS[1, :, k, :])
        if xcB is not None:
            for k in range(KCH):
                nc.sync.dma_start(xc_t[:, k, 512:SC], xcB[:, k, :])
        if cfg.nmask and cfg.nmask_q0 < cfg.nmask:
            nc.sync.dma_start(
                cm_t[:, cfg.nmask_q0 :], cmask[:, cfg.nmask_q0 :]
            )
        nc.sync.dma_start(wo_t[:], wof[:])
        for k4 in range(0, KCH, 2):
            nc.sync.dma_start(xt[:, 2, k4 : k4 + 2, :], xS[2, :, k4 : k4 + 2, :])
        for k4 in range(0, KCH, 2):
            nc.sync.dma_start(xt[:, 3, k4 : k4 + 2, :], xS[3, :, k4 : k4 + 2, :])

    def proj_q(jt, c):
        """one [128, 512] tile of qhT: out partition=j, free=s."""
        ps = ps_proj.tile([128, QT], f32, tag="proj")
        for k in range(KCH):
            nc.tensor.matmul(
                ps[:],
                wq_t[:, jt, k, :],
                xt[:, c, k, :],
                start=(k == 0),
                stop=(k == KCH - 1),
            )
            if k % 2 == 1:
                yield
        with nc.allow_low_precision(reason="bf16 activations"):
            nc.vector.tensor_scalar_add(
                qh_t[:, jt, c * QT : (c + 1) * QT], ps[:], bq_t[:, jt : jt + 1]
            )

    def proj_k(jt, ct):
        """one column tile of khT over compacted keys."""
        c0 = ct * 512
        w = min(512, SC - c0)
        ps = ps_proj.tile([128, QT], f32, tag="proj")
        for k in range(KCH):
            nc.tensor.matmul(
                ps[:, 0:w],
                wk_t[:, jt, k, :],
                xc_t[:, k, c0 : c0 + w],
                start=(k == 0),
                stop=(k == KCH - 1),
            )
            if k % 2 == 1:
                yield
        with nc.allow_low_precision(reason="bf16 activations"):
            nc.vector.tensor_scalar_add(
                kh_t[:, jt, c0 : c0 + w], ps[:, 0:w], bk_t[:, jt : jt + 1]
            )

    def proj_v(sb):
        """one compacted s-block of vh: out partition=s, free=[4 heads x 64]."""
        ps = ps_proj.tile([128, HC], f32, tag="proj")
        for k in range(KCH):
            nc.tensor.matmul(
                ps[:],
                xc_t[:, k, sb * 128 : (sb + 1) * 128],
                wv_t[:, k, :],
                start=(k == 0),
                stop=(k == KCH - 1),
            )
            if k % 2 == 1:
                yield
        with nc.allow_low_precision(reason="bf16 activations"):
            nc.vector.tensor_tensor(
                vh_t[:, sb, :, 0:HD],
                ps[:].rearrange("p (h d) -> p h d", h=HPC),
                bvb[:].rearrange("p (h d) -> p h d", h=HPC),
                mybir.AluOpType.add,
            )

    heads_sb = {}

    def stage_phase1(qi, pair, ot0, ot1):
        """ACT/DVE-only half of normalization: reciprocal of the denominator
        rows and PSUM->SBUF copies of the head outputs (frees the ot banks).
        No PE instructions, so the tensor engine never waits on this chain."""
        raw = stage_pool.tile([128, QT], f32, tag="raw", bufs=4)
        den_w = stage_pool.tile([1, 2, QT], f32, tag="denw")
        # drain the ot banks on BOTH ACT and DVE so neither queue blocks the
        # next pair's exps for long and the banks free as fast as possible
        nc.scalar.copy(den_w[0:1, 0], ot0[HD : HD + 1, :])
        nc.vector.tensor_copy(den_w[0:1, 1], ot1[HD : HD + 1, :])
        nc.scalar.copy(raw[0:HD, :], ot0[0:HD, :])
        nc.vector.tensor_copy(raw[HD : 2 * HD, :], ot1[0:HD, :])
        den_r = stage_pool.tile([1, 2, QT], f32, tag="den")
        den_b = stage_pool.tile([1, 2, QT], bf16, tag="denb")
        with nc.allow_low_precision(reason="approx reciprocal, bf16 denoms"):
            nc.vector.reciprocal_approx_fast(
                den_r[0:1, :, :].rearrange("o a q -> o (a q)"),
                den_w[0:1, :, :].rearrange("o a q -> o (a q)"),
            )
            nc.vector.tensor_copy(den_b[:], den_r[:])
        return raw, den_b

    def stage_phase2(qi, pair, raw, den_b):
        """PE broadcast of the reciprocal denominators (col-tiled pair)
        + normalize multiply into the bf16 heads tile the O-projection
        consumes. Deferred into the next pair's kt-loop."""
        bcst = ps_st.tile([128, QT], f32, tag="st")
        nc.tensor.matmul(
            bcst[0:HD, :], ones_b[0:1, 0:HD], den_b[0:1, 0, :],
            start=True, stop=True,
        )
        nc.tensor.matmul(
            bcst[HD:128, :], ones_b[0:1, 0:HD], den_b[0:1, 1, :],
            start=True, stop=True,
        )
        h = heads_pool.tile([128, QT], bf16, tag="heads", name=f"h{qi}_{pair}")
        with nc.allow_low_precision(reason="bf16 staging"):
            nc.vector.tensor_tensor(
                h[:], raw[:], bcst[:], mybir.AluOpType.mult
            )
        heads_sb[(qi, pair)] = h

    def make_finish(qi, pair, raw, den_r):
        def fin():
            stage_phase2(qi, pair, raw, den_r)
        return fin

    def attention_qtile(qi, finishq, filler=None, defer_last=False):
        q0 = qi * QT
        nk = cfg.nktq[qi]
        for pair in range(2):
            ot0 = ps_ot.tile([HD + 1, QT], f32, tag="ot")
            ot1 = ps_ot.tile([HD + 1, QT], f32, tag="ot")
            ots = (ot0, ot1)
            for kt in range(nk):
                if filler is not None:
                    filler()
                if kt == min(3, nk - 1) and finishq:
                    finishq.pop(0)()
                d0 = cfg.d0[qi][kt]
                st = ps_st.tile([128, 2, QT], f32, tag="st")
                for hh in range(2):
                    nc.tensor.matmul(
                        st[:, hh, d0:QT],
                        kh_t[hh * 64 : hh * 64 + 64, pair, kt * 128 : kt * 128 + 128],
                        qh_t[hh * 64 : hh * 64 + 64, pair, q0 + d0 : q0 + QT],
                        start=True,
                        stop=True,
                    )
                probs = probs_pool.tile([128, 2, QT], bf16, tag="probs")
                with nc.allow_low_precision(reason="bf16 probs"):
                    nc.scalar.activation(
                        probs[:, :, d0:QT],
                        st[:, :, d0:QT],
                        mybir.ActivationFunctionType.Exp,
                        bias=padb_t[:, kt : kt + 1],
                        scale=float(SCALE),
                    )
                mi = cfg.mask_idx.get((qi, kt))
                if mi is not None:
                    # causal boundary mask: bf16 0/1 multiply on the probs
                    # (host-precomputed; exp of an unmasked future score is
                    # at most ~e^8, no overflow before the zeroing)
                    with nc.allow_low_precision(reason="bf16 probs"):
                        nc.vector.tensor_tensor(
                            probs[:, :, d0:QT],
                            probs[:, :, d0:QT],
                            cm_t[:, mi, d0:QT].rearrange(
                                "p (o n) -> p o n", o=1
                            ).broadcast_to([128, 2, QT - d0]),
                            mybir.AluOpType.mult,
                        )
                for hh in range(2):
                    h = 2 * pair + hh
                    nc.tensor.matmul(
                        ots[hh][:, d0:QT],
                        vh_t[:, kt, h, :],
                        probs[:, hh, d0:QT],
                        start=(kt == 0),
                        stop=(kt == nk - 1),
                    )
            if pair == 1 and defer_last:
                return ot0, ot1
            raw, den_r = stage_phase1(qi, pair, ot0, ot1)
            finishq.append(make_finish(qi, pair, raw, den_r))

    def oproj_tile(c):
        """Full-width O-projection partial for q-tile c from this core's own
        normalized heads (row-parallel Wo; host sums the 4 partials)."""
        while finishq and ((c, 0) not in heads_sb or (c, 1) not in heads_sb):
            finishq.pop(0)()
        for jt in range(NJT):
            ps = ps_proj.tile([128, QT], f32, tag="proj")
            nc.tensor.matmul(
                ps[:], wo_t[:, 0, jt, :], heads_sb[(c, 0)][:],
                start=True, stop=False,
            )
            nc.tensor.matmul(
                ps[:], wo_t[:, 1, jt, :], heads_sb[(c, 1)][:],
                start=False, stop=True,
            )
            osb = outsb_pool.tile([128, QT], bf16, tag="osb")
            with nc.allow_low_precision(reason="bf16 output partials"):
                nc.vector.tensor_copy(osb[:], ps[:])
            nc.sync.dma_start(out[c, jt], osb[:])
            yield

    # ---- emission: projections + O-proj finely interleaved with attention -
    def units_for(stage):
        """stage 0: prereqs of attention(0); stage qi+1: work to interleave
        during attention(qi) (prereqs of qi+1, plus oproj(qi-1))."""
        u = []
        if stage == 0:
            for jt in range(2):
                u.append(lambda jt=jt: proj_k(jt, 0))
            for sb in range(cfg.nktq[0]):
                u.append(lambda sb=sb: proj_v(sb))
            for jt in range(2):
                u.append(lambda jt=jt: proj_q(jt, 0))
            return u
        qi = stage - 1  # currently-running attention tile
        if qi + 1 < NQT:
            if qi + 1 < NCT:  # k-proj tile qi+1 (cols beyond 512*(qi+1))
                for jt in range(2):
                    u.append(lambda jt=jt, ct=qi + 1: proj_k(jt, ct))
            for sb in range(cfg.nktq[qi], cfg.nktq[qi + 1]):
                u.append(lambda sb=sb: proj_v(sb))
            for jt in range(2):
                u.append(lambda jt=jt, c=qi + 1: proj_q(jt, c))
        if qi >= 1:
            u.append(lambda c=qi - 1: oproj_tile(c))
        return u

    class Filler:
        def __init__(self, units, budget, skip=0):
            self.units = list(units)
            self.gen = None
            self.budget = budget
            self.skip = skip

        def __call__(self):
            if self.skip > 0:
                self.skip -= 1
                return
            for _ in range(self.budget):
                if self.gen is None:
                    if not self.units:
                        return
                    self.gen = self.units.pop(0)()
                try:
                    next(self.gen)
                except StopIteration:
                    self.gen = None

        def flush(self):
            while self.units or self.gen is not None:
                if self.gen is None:
                    self.gen = self.units.pop(0)()
                for _ in self.gen:
                    pass
                self.gen = None

    # stage 0 prereqs, with bulk loads triggered after the first k-proj
    init_units = units_for(0)
    first = Filler([init_units[0]], 1)
    first.flush()
    bulk_loads()
    Filler(init_units[1:], 1).flush()

    finishq = []
    last_ots = None
    for qi in range(NQT):
        pending = units_for(qi + 1)
        n_att = 2 * cfg.nktq[qi]
        total_steps = len(pending) * 5
        # when the only pending work is an O-projection, hold it back until
        # the deferred phase2 of its second head pair has been popped
        skip = 4 if (pending and len(pending) == 1 and qi >= 1) else 0
        budget = max(1, -(-total_steps // max(1, n_att - skip)))
        filler = Filler(pending, budget, skip=skip)
        last_ots = attention_qtile(
            qi, finishq, filler, defer_last=(qi == NQT - 1)
        )
        filler.flush()

    # tail: tile-3 pair-1 normalization, then its O-projection. The first
    # head-pair's contribution for 4 column tiles is pre-started into spare
    # PSUM slots so the PE isn't idle during the reciprocal chain. (The st
    # pool must stay untouched here: phase2's bcst allocates from it.)
    raw, den_r = stage_phase1(NQT - 1, 1, *last_ots)
    while finishq:
        finishq.pop(0)()
    c3 = NQT - 1
    pre = []
    for jt in range(4):
        pool_ = ps_proj if jt < 2 else ps_ot
        tag_ = "proj" if jt < 2 else "ot"
        ps = pool_.tile([128, QT], f32, tag=tag_, name=f"otail{jt}")
        nc.tensor.matmul(
            ps[:], wo_t[:, 0, jt, :], heads_sb[(c3, 0)][:],
            start=True, stop=False,
        )
        pre.append(ps)
    stage_phase2(c3, 1, raw, den_r)
    for jt in range(NJT):
        if jt < 4:
            ps = pre[jt]
        else:
            ps = ps_proj.tile([128, QT], f32, tag="proj")
            nc.tensor.matmul(
                ps[:], wo_t[:, 0, jt, :], heads_sb[(c3, 0)][:],
                start=True, stop=False,
            )
        nc.tensor.matmul(
            ps[:], wo_t[:, 1, jt, :], heads_sb[(c3, 1)][:],
            start=False, stop=True,
        )
        osb = outsb_pool.tile([128, QT], bf16, tag="osb")
        with nc.allow_low_precision(reason="bf16 output partials"):
            nc.vector.tensor_copy(osb[:], ps[:])
        nc.sync.dma_start(out[c3, jt], osb[:])

    for p in reversed(ctx_pools):
        p.__exit__(None, None, None)


# ---- host-side marshalling ----------------------------------------------


def _bf16(a):
    import ml_dtypes

    return np.ascontiguousarray(
        np.asarray(a, dtype=np.float32).astype(ml_dtypes.bfloat16)
    )


def _wswizzle(WT):
    """[D, 256] -> [128, 2, KCH, 128]: w[p, jt, k, j] = WT[k*128+p, jt*128+j]."""
    return WT.reshape(KCH, 128, 2, 128).transpose(1, 2, 0, 3)


def make_inputs(q, pad_mask, Wq, bq, Wk, bk, Wv, bv, Wo, bo, cfg=None):
    """Build the 8 per-core input maps from full inputs."""
    if cfg is None:
        cfg = Cfg(pad_mask)
    SC, NKTC = cfg.sc, cfg.nktc
    in_maps = []
    xSs, xcAs, xcBs, padbs, cms = [], [], [], [], []
    for b in range(B):
        xb = np.asarray(q[b], dtype=np.float32)
        xSs.append(
            _bf16(xb.T.reshape(KCH, 128, NQT, QT).transpose(2, 1, 0, 3))
        )
        keys = cfg.keys[b]
        scb = cfg.scb[b]
        xcT = np.zeros((D, SC), np.float32)
        xcT[:, :scb] = xb[keys].T
        xc = xcT.reshape(KCH, 128, SC).transpose(1, 0, 2)
        xcAs.append(_bf16(xc[:, :, 0 : min(SC, 512)]))
        xcBs.append(_bf16(xc[:, :, 512:SC]) if SC > 512 else None)
        jj = np.arange(SC)
        padbs.append(
            np.ascontiguousarray(
                np.where(jj < scb, np.float32(0), np.float32(NEG))
                .reshape(NKTC, 128)
                .T
            )
        )
        if cfg.nmask:
            cm = np.zeros((128, cfg.nmask, QT), np.float32)
            qf = np.arange(QT)
            for i, (qi, kt) in enumerate(cfg.mask_order):
                j = kt * 128 + np.arange(128)
                valid = j < scb
                pos = keys[np.minimum(j, scb - 1)]
                cm[:, i, :] = np.where(
                    valid[:, None] & (pos[:, None] > qi * QT + qf[None, :]),
                    np.float32(0),
                    np.float32(1),
                )
            cms.append(_bf16(cm))
        else:
            cms.append(None)

    WoT = np.ascontiguousarray(np.asarray(Wo, dtype=np.float32).T)  # [d, j]
    for core in range(8):
        b, r = divmod(core, GROUP)
        sl = slice(r * HC, (r + 1) * HC)
        # wof rows: chunk pair holds rows hh*64+dd of heads 2*pair+hh
        rows = np.array(
            [
                (r * HPC + 2 * pair + hh) * HD + dd
                for pair in range(2)
                for hh in range(2)
                for dd in range(HD)
            ],
            dtype=np.int64,
        )
        wo4 = WoT[rows].reshape(2, 128, NJT, 128).transpose(1, 0, 2, 3)
        im = {
            "xS": xSs[b],
            "xcA": xcAs[b],
            "wqf": _bf16(_wswizzle(np.asarray(Wq, np.float32)[sl, :].T)),
            "wkf": _bf16(_wswizzle(np.asarray(Wk, np.float32)[sl, :].T)),
            "wvf": _bf16(
                np.asarray(Wv, np.float32)[sl, :].T
                .reshape(KCH, 128, HC).transpose(1, 0, 2)
            ),
            "wof": _bf16(wo4),
            "bq": np.ascontiguousarray(np.asarray(bq)[sl].reshape(2, 128).T),
            "bk": np.ascontiguousarray(np.asarray(bk)[sl].reshape(2, 128).T),
            "bv": _bf16(np.asarray(bv)[sl].reshape(1, HC)),
            "padb": padbs[b],
        }
        if SC > 512:
            im["xcB"] = xcBs[b]
        if cfg.nmask:
            im["cmask"] = cms[b]
        in_maps.append(im)
    return in_maps


def assemble_output(results, bo):
    full = np.zeros((B, S, D), dtype=np.float32)
    for core in range(8):
        b, _ = divmod(core, GROUP)
        o = np.asarray(results[core]["out"], dtype=np.float32)
        # out[c, jt, j, q] = partial for row c*512+q, col jt*128+j
        full[b] += o.transpose(0, 3, 1, 2).reshape(S, D)
    full += np.asarray(bo, dtype=np.float32)[None, None, :]
    return full


_NC_CACHE = [None, None]  # [cfg.key, nc]


def _get_nc(cfg):
    if _NC_CACHE[0] != cfg.key:
        _NC_CACHE[1] = build(cfg)
        _NC_CACHE[0] = cfg.key
    return _NC_CACHE[1]


def kernel(**inputs):
    """Full-input MHA forward. inputs: q, pad_mask, Wq, bq, Wk, bk, Wv, bv,
    Wo, bo (as produced by setup_inputs). Returns [B, S, D] float32."""
    inputs = {k: np.asarray(v) for k, v in inputs.items()}
    cfg = Cfg(inputs["pad_mask"])
    nc = _get_nc(cfg)
    in_maps = make_inputs(**inputs, cfg=cfg)
    res = run_bass_kernel_spmd(nc, in_maps, list(range(8)))
    return assemble_output(res.results, inputs["bo"])


# revision 40
# speedup vs baseline: 1.0188x; 1.0188x over previous
"""Multi-head attention (B=2, S=2048, D=1024, H=16, causal + key-pad mask)
as an 8-core Trainium2 Bass/Tile SPMD kernel.

Sharding: data parallel over the 2 batches (4 cores each); within a batch
group, tensor parallel over heads (4 heads/core) for the QKV projections and
attention. The O-projection is ROW-parallel: each core multiplies its 4
normalized head outputs by its 256 rows of Wo, producing a full-width
[S, 1024] partial sum; the host adds the 4 partials per batch (plus bo).
No device collectives at all.

Key compaction: the pad mask kills ~half the keys, and masked keys contribute
exactly 0 to softmax (exp(-1e9/8) underflows) in the reference too. The host
compacts K/V work to the unmasked key positions (padded to a multiple of 128,
exp-bias NEG on the padding), roughly halving the QK/AV matmuls, the exp
work, and the K/V projections. Causal masking in compacted key space is
data-dependent, so the host precomputes NEG/0 mask tiles for the few key
blocks that straddle each q-tile's causal boundary; fully-past blocks are
never emitted, fully-valid blocks need no mask.

All matmul operands are bf16 (fp32 accumulation in PSUM). Softmax skips
max-subtraction (scores are O(5) here), applies the key-pad mask through the
exp bias and the causal boundary masks via DVE adds. Softmax denominators
ride along as a ones-column in the V operand; normalization uses the fast
approximate DVE reciprocal and an f32r PE ones-broadcast.

Startup: the first projection's inputs are DMA'd first; bulk loads are
triggered from the vector engine's queue after the first projection's bias
add, so they cannot steal HBM bandwidth from the critical path.

self-contained: includes a workaround for the walrus per-instruction
sync-wait limit and an NTFF-profile hook shim.
"""
import sys
import types

import numpy as np

import bass_rust
import concourse.bass as bass
import concourse.mybir as mybir
import concourse.tile as tile


# ---- walrus sync-wait limit workaround ----------------------------------
# This walrus build rejects instructions carrying more than one sem wait
# ("Too many sync wait commands"). Tile emits multi-wait instructions (the
# final drain, matmuls waiting on several DMA queues). Split excess waits
# onto same-engine NoOps placed immediately before the instruction --
# serial waits on one sequencer are semantically identical.
_WSPLIT_COUNTER = [0]


def _split_excess_waits(nc, limit=1):
    for fn in nc.m.functions:
        for bb in fn.blocks:
            out = []
            changed = False
            for inst in bb.instructions:
                si = inst.sync_info
                waits = list(si.on_wait) if si is not None and si.on_wait else []
                if len(waits) > limit:
                    extra, keep = waits[:-limit], waits[-limit:]
                    for s in range(0, len(extra), limit):
                        _WSPLIT_COUNTER[0] += 1
                        nop = mybir.InstNoOp(
                            name=f"I-wsplit-{_WSPLIT_COUNTER[0]}", ins=[], outs=[]
                        )
                        nop.engine = inst.engine
                        nop.sync_info = bass_rust.SyncInfo(
                            on_wait=extra[s : s + limit], on_update=[]
                        )
                        out.append(nop)
                    si.on_wait = keep
                    changed = True
                out.append(inst)
            if changed:
                bb.instructions = out


def _install_tile_patch():
    if getattr(tile.TileContext, "_wait_split_patched", False):
        return
    orig_exit = tile.TileContext.__exit__

    def __exit__(self, exc_type, exc_val, exc_tb):
        r = orig_exit(self, exc_type, exc_val, exc_tb)
        if exc_type is None:
            _split_excess_waits(self.nc)
        return r

    tile.TileContext.__exit__ = __exit__
    tile.TileContext._wait_split_patched = True


_install_tile_patch()


# ---- NTFF profile hook shim (axon deployments missing antenv.axon_hooks) --
def _install_ntff_hook():
    try:
        import antenv.axon_hooks  # noqa: F401
        return
    except ImportError:
        pass
    try:
        from trn_agent_boot.trn_boot import _ntff_profile_via_ctypes

        hook = _ntff_profile_via_ctypes("/opt/axon/libaxon_pjrt.so")
    except Exception:
        hook = None
    m = types.ModuleType("antenv.axon_hooks")
    m.get_axon_ntff_profile_hook = lambda: hook
    m.set_axon_ntff_profile_hook = lambda h: None
    sys.modules["antenv.axon_hooks"] = m


_install_ntff_hook()

import concourse.bass_utils as bass_utils  # noqa: E402
from concourse.bass_utils import run_bass_kernel_spmd  # noqa: E402


# note: --enable-ldw-opt=true was tried here and crashes this walrus build's
# codegen (visitInstLdweights, CoreV3GenImpl.cpp:694) — it is off for a
# reason; LDWEIGHTS overlap must come from instruction scheduling instead.

f32 = mybir.dt.float32
f32r = mybir.dt.float32r
bf16 = mybir.dt.bfloat16

B, S, D, H, HD = 2, 2048, 1024, 16, 64
HPC, GROUP = 4, 4          # heads per core, cores per batch
HC = HPC * HD              # 256 projection cols per core
NQT = S // 512             # 4 q-tiles
QT = 512                   # q-tile width
NJT = D // 128             # 8 output column tiles (full width, row-parallel)
SCALE = 1.0 / np.sqrt(HD)  # 0.125
NEG = -1.0e9
KCH = D // 128             # 8 contraction chunks


class Cfg:
    """Compile-time attention geometry derived from the runtime pad_mask."""

    def __init__(self, pad_mask):
        pad_mask = np.asarray(pad_mask)
        self.keys = [np.flatnonzero(~pad_mask[b]) for b in range(B)]
        self.scb = [len(k) for k in self.keys]
        self.nktc = -(-max(self.scb) // 128)
        self.sc = self.nktc * 128
        cnt = [
            [int((self.keys[b] < (qi + 1) * QT).sum()) for qi in range(NQT)]
            for b in range(B)
        ]
        self.nktq = [
            max(-(-cnt[b][qi] // 128) for b in range(B)) for qi in range(NQT)
        ]
        minpos, maxpos = [], []
        for kt in range(self.nktc):
            mn, mx = S, -1
            for b in range(B):
                lo, hi = kt * 128, min(kt * 128 + 128, self.scb[b])
                if lo < hi:
                    mn = min(mn, int(self.keys[b][lo]))
                    mx = max(mx, int(self.keys[b][hi - 1]))
            minpos.append(mn)
            maxpos.append(mx)
        self.d0 = [
            [max(0, minpos[kt] - qi * QT) for kt in range(self.nktq[qi])]
            for qi in range(NQT)
        ]
        self.mask_order = []          # [(qi, kt)]
        self.mask_idx = {}
        for qi in range(NQT):
            for kt in range(self.nktq[qi]):
                if maxpos[kt] > qi * QT:
                    self.mask_idx[(qi, kt)] = len(self.mask_order)
                    self.mask_order.append((qi, kt))
        self.nmask = len(self.mask_order)
        self.nct = -(-self.sc // 512)  # k-proj column tiles
        # masks needed before attention(qi) starts: index of first mask of qi>0
        self.nmask_q0 = sum(1 for (qi, _) in self.mask_order if qi == 0)
        self.key = (
            self.sc,
            tuple(self.nktq),
            tuple(tuple(r) for r in self.d0),
            tuple(self.mask_order),
        )


def build(cfg):
    nc = bass.Bass()
    dp = nc.declare_dram_parameter
    # xS[c, p, k, j] = x[c*512+j, k*128+p]: contiguous 8KiB per partition.
    xS = dp("xS", [NQT, 128, KCH, QT], bf16, isOutput=False)
    # xcA/xcB[p, k, j] = x[keys[j'], k*128+p] over compacted keys.
    xcA = dp("xcA", [128, KCH, min(cfg.sc, 512)], bf16, isOutput=False)
    if cfg.sc > 512:
        xcB = dp("xcB", [128, KCH, cfg.sc - 512], bf16, isOutput=False)
    else:
        xcB = None
    # w*f[p, jt, k, j] = W.T[k*128+p, jt*128+j] over this core's 256 cols.
    wqf = dp("wqf", [128, 2, KCH, 128], bf16, isOutput=False)
    wkf = dp("wkf", [128, 2, KCH, 128], bf16, isOutput=False)
    wvf = dp("wvf", [128, KCH, HC], bf16, isOutput=False)
    # wof[p, pair, jt, j]: Wo rows for this core's heads, pair-chunked.
    wof = dp("wof", [128, 2, NJT, 128], bf16, isOutput=False)
    bq = dp("bq", [128, 2], f32, isOutput=False)
    bk = dp("bk", [128, 2], f32, isOutput=False)
    bv = dp("bv", [1, HC], bf16, isOutput=False)
    padb = dp("padb", [128, cfg.nktc], f32, isOutput=False)
    if cfg.nmask:
        cmask = dp("cmask", [128, cfg.nmask, QT], bf16, isOutput=False)
    else:
        cmask = None
    out = dp("out", [NQT, NJT, 128, QT], bf16, isOutput=True)

    with tile.TileContext(nc) as tc:
        _body(nc, tc, cfg, locals())
    # populate .instr bytes for extended-inst ISA ops (custom DVE reciprocal);
    # Bacc.compile runs this pass but the raw-Bass path does not.
    from concourse.library_overlay import lower_extended_insts

    lower_extended_insts(nc)
    return nc


def _body(nc, tc, cfg, t):
    xS, xcA, xcB, wqf, wkf, wvf, wof = (
        t["xS"], t["xcA"], t["xcB"], t["wqf"], t["wkf"], t["wvf"], t["wof"]
    )
    bq, bk, bv, padb, cmask, out = (
        t["bq"], t["bk"], t["bv"], t["padb"], t["cmask"], t["out"]
    )
    SC, NKTC, NCT = cfg.sc, cfg.nktc, cfg.nct

    ctx_pools = []

    def pool(name, bufs, space="SBUF"):
        p = tc.tile_pool(name=name, bufs=bufs, space=space)
        ctx_pools.append(p)
        return p.__enter__()

    const = pool("const", 1)
    probs_pool = pool("probs", 4)
    stage_pool = pool("stage", 2)
    heads_pool = pool("heads", 4)
    outsb_pool = pool("outsb", 3)

    ps_proj = pool("ps_proj", 2, space="PSUM")
    ps_st = pool("ps_st", 2, space="PSUM")
    ps_ot = pool("ps_ot", 2, space="PSUM")

    ones_f = const.tile([128, 128], f32)
    nc.any.memset(ones_f[:], 1.0)
    ones_b = const.tile([1, 128], bf16)
    with nc.allow_low_precision(reason="exact small ints in bf16"):
        nc.vector.tensor_copy(ones_b[0:1, :], ones_f[0:1, :])

    # HAM warmup: the PE would otherwise idle ~15us waiting for the input
    # DMAs and the first ~3.4us of real matmuls would run at the 1.2 GHz
    # cold clock. Burn the wait on dummy matmuls (results never read) so the
    # activity monitor reaches 8/8 before the first projection.
    zdum = const.tile([128, QT], bf16)
    nc.vector.memset(zdum[:], 0.0)
    for i in range(20):
        psd = ps_st.tile([128, QT], f32, tag="st", name=f"psd{i}")
        nc.tensor.matmul(psd[:], zdum[:, 0:128], zdum[:], start=True, stop=True)
    # preload the ACT exp table during the input-DMA wait — otherwise the
    # 1.3us ACT_TABLE_LOAD serializes in front of the first real exp
    warm_e = const.tile([1, 2], f32)
    nc.scalar.activation(
        warm_e[:], ones_f[0:1, 0:2], mybir.ActivationFunctionType.Exp,
        scale=1.0,
    )

    # ---- small consts first (tiny DMAs) ---------------------------------
    bq_t = const.tile([128, 2], f32)
    nc.sync.dma_start(bq_t[:], bq[:])
    bk_t = const.tile([128, 2], f32)
    nc.sync.dma_start(bk_t[:], bk[:])
    padb_t = const.tile([128, NKTC], f32)
    nc.sync.dma_start(padb_t[:], padb[:])
    bv_row = const.tile([1, HC], bf16)
    nc.sync.dma_start(bv_row[:], bv[:])

    # ---- critical-path inputs in dependency order ------------------------
    xt = const.tile([128, NQT, KCH, QT], bf16)   # full x, s-slice-major
    xc_t = const.tile([128, KCH, SC], bf16)      # compacted-key x
    wq_t = const.tile([128, 2, KCH, 128], bf16)
    wk_t = const.tile([128, 2, KCH, 128], bf16)
    wv_t = const.tile([128, KCH, HC], bf16)
    wo_t = const.tile([128, 2, NJT, 128], bf16)
    if cfg.nmask:
        cm_t = const.tile([128, cfg.nmask, QT], bf16)
    else:
        cm_t = None

    # First wave, chunked so the k-projection's inputs own every DMA ring:
    # wk jt0/jt1 (k-proj both jt) + xcA in 128-col slices.
    CA = min(SC, 512)
    for k4 in range(0, KCH, 4):
        nc.sync.dma_start(wk_t[:, 0, k4 : k4 + 4], wkf[:, 0, k4 : k4 + 4])
    for k4 in range(0, KCH, 4):
        nc.sync.dma_start(wk_t[:, 1, k4 : k4 + 4], wkf[:, 1, k4 : k4 + 4])
    for c4 in range(0, CA, 128):
        nc.sync.dma_start(
            xc_t[:, :, c4 : c4 + 128], xcA[:, :, c4 : c4 + 128]
        )
    # Second wave: q-projection inputs. Gated on wave-1 completion (the
    # dummy copy reads the last xcA chunk and writes a wq cell, and the SP
    # queue dispatches DMA triggers in order) so the k-projection's inputs
    # get the full HBM bandwidth and the PE can start earlier.
    with nc.allow_low_precision(reason="dummy gate, overwritten"):
        nc.vector.tensor_copy(wq_t[:, 0, 0, 0:1], xc_t[:, KCH - 1, CA - 1 : CA])
    nc.sync.dma_start(wq_t[:, 0], wqf[:, 0])
    nc.sync.dma_start(wq_t[:, 1], wqf[:, 1])
    nc.sync.dma_start(xt[:, 0], xS[0])
    nc.sync.dma_start(wv_t[:], wvf[:])
    if cfg.nmask and cfg.nmask_q0:
        nc.sync.dma_start(cm_t[:, 0 : cfg.nmask_q0], cmask[:, 0 : cfg.nmask_q0])

    bvb = const.tile([128, HC], f32)
    bv_ps = ps_proj.tile([128, HC], f32, tag="proj")
    nc.tensor.matmul(bv_ps[:], ones_b[:], bv_row[:], start=True, stop=True)
    nc.vector.tensor_copy(bvb[:], bv_ps[:])

    # projection outputs
    qh_t = const.tile([128, 2, S], bf16)    # qhT: [j-in-tile, j-tile, s]
    kh_t = const.tile([128, 2, SC], bf16)
    vh_t = const.tile([128, NKTC, HPC, HD + 1], bf16)
    with nc.allow_low_precision(reason="exact small ints in bf16"):
        nc.vector.tensor_copy(
            vh_t[:, :, :, HD : HD + 1].rearrange("p t h o -> p (t h) o"),
            ones_f[:, 0 : NKTC * HPC].rearrange("p (f o) -> p f o", o=1),
        )

    def bulk_loads():
        """Deferred loads. The dummy copy makes the first bulk DMA depend on
        the last critical load (xS[0]); since the SP queue dispatches DMA
        triggers in program order, every bulk transfer queues behind it and
        cannot starve the critical-path loads of HBM bandwidth."""
        with nc.allow_low_precision(reason="dummy gate, overwritten"):
            nc.vector.tensor_copy(
                xt[:, 1, 0, 0:1], xt[:, 0, KCH - 1, QT - 1 : QT]
            )
        nc.sync.dma_start(xt[:, 1], xS[1])
        if xcB is not None:
            nc.sync.dma_start(xc_t[:, :, 512:SC], xcB[:])
        if cfg.nmask and cfg.nmask_q0 < cfg.nmask:
            nc.sync.dma_start(
                cm_t[:, cfg.nmask_q0 :], cmask[:, cfg.nmask_q0 :]
            )
        nc.sync.dma_start(wo_t[:], wof[:])
        nc.sync.dma_start(xt[:, 2], xS[2])
        nc.sync.dma_start(xt[:, 3], xS[3])

    def proj_q(jt, c):
        """one [128, 512] tile of qhT: out partition=j, free=s."""
        ps = ps_proj.tile([128, QT], f32, tag="proj")
        for k in range(KCH):
            nc.tensor.matmul(
                ps[:],
                wq_t[:, jt, k, :],
                xt[:, c, k, :],
                start=(k == 0),
                stop=(k == KCH - 1),
            )
            if k % 2 == 1:
                yield
        with nc.allow_low_precision(reason="bf16 activations"):
            nc.vector.tensor_scalar_add(
                qh_t[:, jt, c * QT : (c + 1) * QT], ps[:], bq_t[:, jt : jt + 1]
            )

    def proj_k(jt, ct):
        """one column tile of khT over compacted keys."""
        c0 = ct * 512
        w = min(512, SC - c0)
        ps = ps_proj.tile([128, QT], f32, tag="proj")
        for k in range(KCH):
            nc.tensor.matmul(
                ps[:, 0:w],
                wk_t[:, jt, k, :],
                xc_t[:, k, c0 : c0 + w],
                start=(k == 0),
                stop=(k == KCH - 1),
            )
            if k % 2 == 1:
                yield
        with nc.allow_low_precision(reason="bf16 activations"):
            nc.vector.tensor_scalar_add(
                kh_t[:, jt, c0 : c0 + w], ps[:, 0:w], bk_t[:, jt : jt + 1]
            )

    def proj_v(sb):
        """one compacted s-block of vh: out partition=s, free=[4 heads x 64]."""
        ps = ps_proj.tile([128, HC], f32, tag="proj")
        for k in range(KCH):
            nc.tensor.matmul(
                ps[:],
                xc_t[:, k, sb * 128 : (sb + 1) * 128],
                wv_t[:, k, :],
                start=(k == 0),
                stop=(k == KCH - 1),
            )
            if k % 2 == 1:
                yield
        with nc.allow_low_precision(reason="bf16 activations"):
            nc.vector.tensor_tensor(
                vh_t[:, sb, :, 0:HD],
                ps[:].rearrange("p (h d) -> p h d", h=HPC),
                bvb[:].rearrange("p (h d) -> p h d", h=HPC),
                mybir.AluOpType.add,
            )

    heads_sb = {}

    def stage_phase1(qi, pair, ot0, ot1):
        """ACT/DVE-only half of normalization: reciprocal of the denominator
        rows and PSUM->SBUF copies of the head outputs (frees the ot banks).
        No PE instructions, so the tensor engine never waits on this chain."""
        raw = stage_pool.tile([128, QT], f32, tag="raw", bufs=4)
        den_w = stage_pool.tile([1, 2, QT], f32, tag="denw")
        # drain the ot banks on BOTH ACT and DVE so neither queue blocks the
        # next pair's exps for long and the banks free as fast as possible
        nc.scalar.copy(den_w[0:1, 0], ot0[HD : HD + 1, :])
        nc.vector.tensor_copy(den_w[0:1, 1], ot1[HD : HD + 1, :])
        nc.scalar.copy(raw[0:HD, :], ot0[0:HD, :])
        nc.vector.tensor_copy(raw[HD : 2 * HD, :], ot1[0:HD, :])
        den_r = stage_pool.tile([1, 2, QT], f32, tag="den")
        den_b = stage_pool.tile([1, 2, QT], bf16, tag="denb")
        with nc.allow_low_precision(reason="approx reciprocal, bf16 denoms"):
            nc.vector.reciprocal_approx_fast(
                den_r[0:1, :, :].rearrange("o a q -> o (a q)"),
                den_w[0:1, :, :].rearrange("o a q -> o (a q)"),
            )
            nc.vector.tensor_copy(den_b[:], den_r[:])
        return raw, den_b

    def stage_phase2(qi, pair, raw, den_b):
        """PE broadcast of the reciprocal denominators (col-tiled pair)
        + normalize multiply into the bf16 heads tile the O-projection
        consumes. Deferred into the next pair's kt-loop."""
        bcst = ps_st.tile([128, QT], f32, tag="st")
        nc.tensor.matmul(
            bcst[0:HD, :], ones_b[0:1, 0:HD], den_b[0:1, 0, :],
            start=True, stop=True,
        )
        nc.tensor.matmul(
            bcst[HD:128, :], ones_b[0:1, 0:HD], den_b[0:1, 1, :],
            start=True, stop=True,
        )
        h = heads_pool.tile([128, QT], bf16, tag="heads", name=f"h{qi}_{pair}")
        with nc.allow_low_precision(reason="bf16 staging"):
            nc.vector.tensor_tensor(
                h[:], raw[:], bcst[:], mybir.AluOpType.mult
            )
        heads_sb[(qi, pair)] = h

    def make_finish(qi, pair, raw, den_r):
        def fin():
            stage_phase2(qi, pair, raw, den_r)
        return fin

    def attention_qtile(qi, finishq, filler=None, defer_last=False):
        q0 = qi * QT
        nk = cfg.nktq[qi]
        for pair in range(2):
            ot0 = ps_ot.tile([HD + 1, QT], f32, tag="ot")
            ot1 = ps_ot.tile([HD + 1, QT], f32, tag="ot")
            ots = (ot0, ot1)
            for kt in range(nk):
                if filler is not None:
                    filler()
                if kt == min(3, nk - 1) and finishq:
                    finishq.pop(0)()
                d0 = cfg.d0[qi][kt]
                st = ps_st.tile([128, 2, QT], f32, tag="st")
                for hh in range(2):
                    nc.tensor.matmul(
                        st[:, hh, d0:QT],
                        kh_t[hh * 64 : hh * 64 + 64, pair, kt * 128 : kt * 128 + 128],
                        qh_t[hh * 64 : hh * 64 + 64, pair, q0 + d0 : q0 + QT],
                        start=True,
                        stop=True,
                    )
                probs = probs_pool.tile([128, 2, QT], bf16, tag="probs")
                with nc.allow_low_precision(reason="bf16 probs"):
                    nc.scalar.activation(
                        probs[:, :, d0:QT],
                        st[:, :, d0:QT],
                        mybir.ActivationFunctionType.Exp,
                        bias=padb_t[:, kt : kt + 1],
                        scale=float(SCALE),
                    )
                mi = cfg.mask_idx.get((qi, kt))
                if mi is not None:
                    # causal boundary mask: bf16 0/1 multiply on the probs
                    # (host-precomputed; exp of an unmasked future score is
                    # at most ~e^8, no overflow before the zeroing)
                    with nc.allow_low_precision(reason="bf16 probs"):
                        nc.vector.tensor_tensor(
                            probs[:, :, d0:QT],
                            probs[:, :, d0:QT],
                            cm_t[:, mi, d0:QT].rearrange(
                                "p (o n) -> p o n", o=1
                            ).broadcast_to([128, 2, QT - d0]),
                            mybir.AluOpType.mult,
                        )
                for hh in range(2):
                    h = 2 * pair + hh
                    nc.tensor.matmul(
                        ots[hh][:, d0:QT],
                        vh_t[:, kt, h, :],
                        probs[:, hh, d0:QT],
                        start=(kt == 0),
                        stop=(kt == nk - 1),
                    )
            if pair == 1 and defer_last:
                return ot0, ot1
            raw, den_r = stage_phase1(qi, pair, ot0, ot1)
            finishq.append(make_finish(qi, pair, raw, den_r))

    def oproj_tile(c):
        """Full-width O-projection partial for q-tile c from this core's own
        normalized heads (row-parallel Wo; host sums the 4 partials)."""
        while finishq and ((c, 0) not in heads_sb or (c, 1) not in heads_sb):
            finishq.pop(0)()
        for jt in range(NJT):
            ps = ps_proj.tile([128, QT], f32, tag="proj")
            nc.tensor.matmul(
                ps[:], wo_t[:, 0, jt, :], heads_sb[(c, 0)][:],
                start=True, stop=False,
            )
            nc.tensor.matmul(
                ps[:], wo_t[:, 1, jt, :], heads_sb[(c, 1)][:],
                start=False, stop=True,
            )
            osb = outsb_pool.tile([128, QT], bf16, tag="osb")
            with nc.allow_low_precision(reason="bf16 output partials"):
                nc.vector.tensor_copy(osb[:], ps[:])
            nc.sync.dma_start(out[c, jt], osb[:])
            yield

    # ---- emission: projections + O-proj finely interleaved with attention -
    def units_for(stage):
        """stage 0: prereqs of attention(0); stage qi+1: work to interleave
        during attention(qi) (prereqs of qi+1, plus oproj(qi-1))."""
        u = []
        if stage == 0:
            for jt in range(2):
                u.append(lambda jt=jt: proj_k(jt, 0))
            for jt in range(2):
                u.append(lambda jt=jt: proj_q(jt, 0))
            for sb in range(cfg.nktq[0]):
                u.append(lambda sb=sb: proj_v(sb))
            return u
        qi = stage - 1  # currently-running attention tile
        if qi + 1 < NQT:
            if qi + 1 < NCT:  # k-proj tile qi+1 (cols beyond 512*(qi+1))
                for jt in range(2):
                    u.append(lambda jt=jt, ct=qi + 1: proj_k(jt, ct))
            for sb in range(cfg.nktq[qi], cfg.nktq[qi + 1]):
                u.append(lambda sb=sb: proj_v(sb))
            for jt in range(2):
                u.append(lambda jt=jt, c=qi + 1: proj_q(jt, c))
        if qi >= 1:
            u.append(lambda c=qi - 1: oproj_tile(c))
        return u

    class Filler:
        def __init__(self, units, budget, skip=0):
            self.units = list(units)
            self.gen = None
            self.budget = budget
            self.skip = skip

        def __call__(self):
            if self.skip > 0:
                self.skip -= 1
                return
            for _ in range(self.budget):
                if self.gen is None:
                    if not self.units:
                        return
                    self.gen = self.units.pop(0)()
                try:
                    next(self.gen)
                except StopIteration:
                    self.gen = None

        def flush(self):
            while self.units or self.gen is not None:
                if self.gen is None:
                    self.gen = self.units.pop(0)()
                for _ in self.gen:
                    pass
                self.gen = None

    # stage 0 prereqs, with bulk loads triggered after the first k-proj
    init_units = units_for(0)
    first = Filler([init_units[0]], 1)
    first.flush()
    bulk_loads()
    Filler(init_units[1:], 1).flush()

    finishq = []
    last_ots = None
    for qi in range(NQT):
        pending = units_for(qi + 1)
        n_att = 2 * cfg.nktq[qi]
        total_steps = len(pending) * 5
        # when the only pending work is an O-projection, hold it back until
        # the deferred phase2 of its second head pair has been popped
        skip = 4 if (pending and len(pending) == 1 and qi >= 1) else 0
        budget = max(1, -(-total_steps // max(1, n_att - skip)))
        filler = Filler(pending, budget, skip=skip)
        last_ots = attention_qtile(
            qi, finishq, filler, defer_last=(qi == NQT - 1)
        )
        filler.flush()

    # tail: tile-3 pair-1 normalization, then its O-projection. The first
    # head-pair's contribution for 4 column tiles is pre-started into spare
    # PSUM slots so the PE isn't idle during the reciprocal chain. (The st
    # pool must stay untouched here: phase2's bcst allocates from it.)
    raw, den_r = stage_phase1(NQT - 1, 1, *last_ots)
    while finishq:
        finishq.pop(0)()
    c3 = NQT - 1
    pre = []
    for jt in range(4):
        pool_ = ps_proj if jt < 2 else ps_ot
        tag_ = "proj" if jt < 2 else "ot"
        ps = pool_.tile([128, QT], f32, tag=tag_, name=f"otail{jt}")
        nc.tensor.matmul(
            ps[:], wo_t[:, 0, jt, :], heads_sb[(c3, 0)][:],
            start=True, stop=False,
        )
        pre.append(ps)
    stage_phase2(c3, 1, raw, den_r)
    for jt in range(NJT):
        if jt < 4:
            ps = pre[jt]
        else:
            ps = ps_proj.tile([128, QT], f32, tag="proj")
            nc.tensor.matmul(
                ps[:], wo_t[:, 0, jt, :], heads_sb[(c3, 0)][:],
                start=True, stop=False,
            )
        nc.tensor.matmul(
            ps[:], wo_t[:, 1, jt, :], heads_sb[(c3, 1)][:],
            start=False, stop=True,
        )
        osb = outsb_pool.tile([128, QT], bf16, tag="osb")
        with nc.allow_low_precision(reason="bf16 output partials"):
            nc.vector.tensor_copy(osb[:], ps[:])
        nc.sync.dma_start(out[c3, jt], osb[:])

    for p in reversed(ctx_pools):
        p.__exit__(None, None, None)


# ---- host-side marshalling ----------------------------------------------


def _bf16(a):
    import ml_dtypes

    return np.ascontiguousarray(
        np.asarray(a, dtype=np.float32).astype(ml_dtypes.bfloat16)
    )


def _wswizzle(WT):
    """[D, 256] -> [128, 2, KCH, 128]: w[p, jt, k, j] = WT[k*128+p, jt*128+j]."""
    return WT.reshape(KCH, 128, 2, 128).transpose(1, 2, 0, 3)


def make_inputs(q, pad_mask, Wq, bq, Wk, bk, Wv, bv, Wo, bo, cfg=None):
    """Build the 8 per-core input maps from full inputs."""
    if cfg is None:
        cfg = Cfg(pad_mask)
    SC, NKTC = cfg.sc, cfg.nktc
    in_maps = []
    xSs, xcAs, xcBs, padbs, cms = [], [], [], [], []
    for b in range(B):
        xb = np.asarray(q[b], dtype=np.float32)
        xSs.append(
            _bf16(xb.T.reshape(KCH, 128, NQT, QT).transpose(2, 1, 0, 3))
        )
        keys = cfg.keys[b]
        scb = cfg.scb[b]
        xcT = np.zeros((D, SC), np.float32)
        xcT[:, :scb] = xb[keys].T
        xc = xcT.reshape(KCH, 128, SC).transpose(1, 0, 2)
        xcAs.append(_bf16(xc[:, :, 0 : min(SC, 512)]))
        xcBs.append(_bf16(xc[:, :, 512:SC]) if SC > 512 else None)
        jj = np.arange(SC)
        padbs.append(
            np.ascontiguousarray(
                np.where(jj < scb, np.float32(0), np.float32(NEG))
                .reshape(NKTC, 128)
                .T
            )
        )
        if cfg.nmask:
            cm = np.zeros((128, cfg.nmask, QT), np.float32)
            qf = np.arange(QT)
            for i, (qi, kt) in enumerate(cfg.mask_order):
                j = kt * 128 + np.arange(128)
                valid = j < scb
                pos = keys[np.minimum(j, scb - 1)]
                cm[:, i, :] = np.where(
                    valid[:, None] & (pos[:, None] > qi * QT + qf[None, :]),
                    np.float32(0),
                    np.float32(1),
                )
            cms.append(_bf16(cm))
        else:
            cms.append(None)

    WoT = np.ascontiguousarray(np.asarray(Wo, dtype=np.float32).T)  # [d, j]
    for core in range(8):
        b, r = divmod(core, GROUP)
        sl = slice(r * HC, (r + 1) * HC)
        # wof rows: chunk pair holds rows hh*64+dd of heads 2*pair+hh
        rows = np.array(
            [
                (r * HPC + 2 * pair + hh) * HD + dd
                for pair in range(2)
                for hh in range(2)
                for dd in range(HD)
            ],
            dtype=np.int64,
        )
        wo4 = WoT[rows].reshape(2, 128, NJT, 128).transpose(1, 0, 2, 3)
        im = {
            "xS": xSs[b],
            "xcA": xcAs[b],
            "wqf": _bf16(_wswizzle(np.asarray(Wq, np.float32)[sl, :].T)),
            "wkf": _bf16(_wswizzle(np.asarray(Wk, np.float32)[sl, :].T)),
            "wvf": _bf16(
                np.asarray(Wv, np.float32)[sl, :].T
                .reshape(KCH, 128, HC).transpose(1, 0, 2)
            ),
            "wof": _bf16(wo4),
            "bq": np.ascontiguousarray(np.asarray(bq)[sl].reshape(2, 128).T),
            "bk": np.ascontiguousarray(np.asarray(bk)[sl].reshape(2, 128).T),
            "bv": _bf16(np.asarray(bv)[sl].reshape(1, HC)),
            "padb": padbs[b],
        }
        if SC > 512:
            im["xcB"] = xcBs[b]
        if cfg.nmask:
            im["cmask"] = cms[b]
        in_maps.append(im)
    return in_maps


def assemble_output(results, bo):
    full = np.zeros((B, S, D), dtype=np.float32)
    for core in range(8):
        b, _ = divmod(core, GROUP)
        o = np.asarray(results[core]["out"], dtype=np.float32)
        # out[c, jt, j, q] = partial for row c*512+q, col jt*128+j
        full[b] += o.transpose(0, 3, 1, 2).reshape(S, D)
    full += np.asarray(bo, dtype=np.float32)[None, None, :]
    return full


_NC_CACHE = [None, None]  # [cfg.key, nc]


def _get_nc(cfg):
    if _NC_CACHE[0] != cfg.key:
        _NC_CACHE[1] = build(cfg)
        _NC_CACHE[0] = cfg.key
    return _NC_CACHE[1]


def kernel(**inputs):
    """Full-input MHA forward. inputs: q, pad_mask, Wq, bq, Wk, bk, Wv, bv,
    Wo, bo (as produced by setup_inputs). Returns [B, S, D] float32."""
    inputs = {k: np.asarray(v) for k, v in inputs.items()}
    cfg = Cfg(inputs["pad_mask"])
    nc = _get_nc(cfg)
    in_maps = make_inputs(**inputs, cfg=cfg)
    res = run_bass_kernel_spmd(nc, in_maps, list(range(8)))
    return assemble_output(res.results, inputs["bo"])


# revision 43
# speedup vs baseline: 1.0269x; 1.0080x over previous
"""Multi-head attention (B=2, S=2048, D=1024, H=16, causal + key-pad mask)
as an 8-core Trainium2 Bass/Tile SPMD kernel.

Sharding: data parallel over the 2 batches (4 cores each); within a batch
group, tensor parallel over heads (4 heads/core) for the QKV projections and
attention. The O-projection is ROW-parallel: each core multiplies its 4
normalized head outputs by its 256 rows of Wo, producing a full-width
[S, 1024] partial sum; the host adds the 4 partials per batch (plus bo).
No device collectives at all.

Key compaction: the pad mask kills ~half the keys, and masked keys contribute
exactly 0 to softmax (exp(-1e9/8) underflows) in the reference too. The host
compacts K/V work to the unmasked key positions (padded to a multiple of 128,
exp-bias NEG on the padding), roughly halving the QK/AV matmuls, the exp
work, and the K/V projections. Causal masking in compacted key space is
data-dependent, so the host precomputes NEG/0 mask tiles for the few key
blocks that straddle each q-tile's causal boundary; fully-past blocks are
never emitted, fully-valid blocks need no mask.

All matmul operands are bf16 (fp32 accumulation in PSUM). Softmax skips
max-subtraction (scores are O(5) here), applies the key-pad mask through the
exp bias and the causal boundary masks via DVE adds. Softmax denominators
ride along as a ones-column in the V operand; normalization uses the fast
approximate DVE reciprocal and an f32r PE ones-broadcast.

Startup: the first projection's inputs are DMA'd first; bulk loads are
triggered from the vector engine's queue after the first projection's bias
add, so they cannot steal HBM bandwidth from the critical path.

self-contained: includes a workaround for the walrus per-instruction
sync-wait limit and an NTFF-profile hook shim.
"""
import sys
import types

import numpy as np

import bass_rust
import concourse.bass as bass
import concourse.mybir as mybir
import concourse.tile as tile


# ---- walrus sync-wait limit workaround ----------------------------------
# This walrus build rejects instructions carrying more than one sem wait
# ("Too many sync wait commands"). Tile emits multi-wait instructions (the
# final drain, matmuls waiting on several DMA queues). Split excess waits
# onto same-engine NoOps placed immediately before the instruction --
# serial waits on one sequencer are semantically identical.
_WSPLIT_COUNTER = [0]


def _split_excess_waits(nc, limit=1):
    for fn in nc.m.functions:
        for bb in fn.blocks:
            out = []
            changed = False
            for inst in bb.instructions:
                si = inst.sync_info
                waits = list(si.on_wait) if si is not None and si.on_wait else []
                if len(waits) > limit:
                    extra, keep = waits[:-limit], waits[-limit:]
                    for s in range(0, len(extra), limit):
                        _WSPLIT_COUNTER[0] += 1
                        nop = mybir.InstNoOp(
                            name=f"I-wsplit-{_WSPLIT_COUNTER[0]}", ins=[], outs=[]
                        )
                        nop.engine = inst.engine
                        nop.sync_info = bass_rust.SyncInfo(
                            on_wait=extra[s : s + limit], on_update=[]
                        )
                        out.append(nop)
                    si.on_wait = keep
                    changed = True
                out.append(inst)
            if changed:
                bb.instructions = out


def _install_tile_patch():
    if getattr(tile.TileContext, "_wait_split_patched", False):
        return
    orig_exit = tile.TileContext.__exit__

    def __exit__(self, exc_type, exc_val, exc_tb):
        r = orig_exit(self, exc_type, exc_val, exc_tb)
        if exc_type is None:
            _split_excess_waits(self.nc)
        return r

    tile.TileContext.__exit__ = __exit__
    tile.TileContext._wait_split_patched = True


_install_tile_patch()


# ---- NTFF profile hook shim (axon deployments missing antenv.axon_hooks) --
def _install_ntff_hook():
    try:
        import antenv.axon_hooks  # noqa: F401
        return
    except ImportError:
        pass
    try:
        from trn_agent_boot.trn_boot import _ntff_profile_via_ctypes

        hook = _ntff_profile_via_ctypes("/opt/axon/libaxon_pjrt.so")
    except Exception:
        hook = None
    m = types.ModuleType("antenv.axon_hooks")
    m.get_axon_ntff_profile_hook = lambda: hook
    m.set_axon_ntff_profile_hook = lambda h: None
    sys.modules["antenv.axon_hooks"] = m


_install_ntff_hook()

import concourse.bass_utils as bass_utils  # noqa: E402
from concourse.bass_utils import run_bass_kernel_spmd  # noqa: E402


# note: --enable-ldw-opt=true was tried here and crashes this walrus build's
# codegen (visitInstLdweights, CoreV3GenImpl.cpp:694) — it is off for a
# reason; LDWEIGHTS overlap must come from instruction scheduling instead.

f32 = mybir.dt.float32
f32r = mybir.dt.float32r
bf16 = mybir.dt.bfloat16

B, S, D, H, HD = 2, 2048, 1024, 16, 64
HPC, GROUP = 4, 4          # heads per core, cores per batch
HC = HPC * HD              # 256 projection cols per core
NQT = S // 512             # 4 q-tiles
QT = 512                   # q-tile width
NJT = D // 128             # 8 output column tiles (full width, row-parallel)
SCALE = 1.0 / np.sqrt(HD)  # 0.125
NEG = -1.0e9
KCH = D // 128             # 8 contraction chunks


class Cfg:
    """Compile-time attention geometry derived from the runtime pad_mask."""

    def __init__(self, pad_mask):
        pad_mask = np.asarray(pad_mask)
        self.keys = [np.flatnonzero(~pad_mask[b]) for b in range(B)]
        self.scb = [len(k) for k in self.keys]
        self.nktc = -(-max(self.scb) // 128)
        self.sc = self.nktc * 128
        cnt = [
            [int((self.keys[b] < (qi + 1) * QT).sum()) for qi in range(NQT)]
            for b in range(B)
        ]
        self.nktq = [
            max(-(-cnt[b][qi] // 128) for b in range(B)) for qi in range(NQT)
        ]
        minpos, maxpos = [], []
        for kt in range(self.nktc):
            mn, mx = S, -1
            for b in range(B):
                lo, hi = kt * 128, min(kt * 128 + 128, self.scb[b])
                if lo < hi:
                    mn = min(mn, int(self.keys[b][lo]))
                    mx = max(mx, int(self.keys[b][hi - 1]))
            minpos.append(mn)
            maxpos.append(mx)
        self.d0 = [
            [max(0, minpos[kt] - qi * QT) for kt in range(self.nktq[qi])]
            for qi in range(NQT)
        ]
        self.mask_order = []          # [(qi, kt)]
        self.mask_idx = {}
        for qi in range(NQT):
            for kt in range(self.nktq[qi]):
                if maxpos[kt] > qi * QT:
                    self.mask_idx[(qi, kt)] = len(self.mask_order)
                    self.mask_order.append((qi, kt))
        self.nmask = len(self.mask_order)
        self.nct = -(-self.sc // 512)  # k-proj column tiles
        # masks needed before attention(qi) starts: index of first mask of qi>0
        self.nmask_q0 = sum(1 for (qi, _) in self.mask_order if qi == 0)
        self.key = (
            self.sc,
            tuple(self.nktq),
            tuple(tuple(r) for r in self.d0),
            tuple(self.mask_order),
        )


def build(cfg):
    nc = bass.Bass()
    dp = nc.declare_dram_parameter
    # xS[c, p, k, j] = x[c*512+j, k*128+p]: contiguous 8KiB per partition.
    xS = dp("xS", [NQT, 128, KCH, QT], bf16, isOutput=False)
    # xcA/xcB[p, k, j] = x[keys[j'], k*128+p] over compacted keys.
    xcA = dp("xcA", [128, KCH, min(cfg.sc, 512)], bf16, isOutput=False)
    if cfg.sc > 512:
        xcB = dp("xcB", [128, KCH, cfg.sc - 512], bf16, isOutput=False)
    else:
        xcB = None
    # w*f[p, jt, k, j] = W.T[k*128+p, jt*128+j] over this core's 256 cols.
    wqf = dp("wqf", [128, 2, KCH, 128], bf16, isOutput=False)
    wkf = dp("wkf", [128, 2, KCH, 128], bf16, isOutput=False)
    wvf = dp("wvf", [128, KCH, HC], bf16, isOutput=False)
    # wof[p, pair, jt, j]: Wo rows for this core's heads, pair-chunked.
    wof = dp("wof", [128, 2, NJT, 128], bf16, isOutput=False)
    bq = dp("bq", [128, 2], f32, isOutput=False)
    bk = dp("bk", [128, 2], f32, isOutput=False)
    bv = dp("bv", [1, HC], bf16, isOutput=False)
    padb = dp("padb", [128, cfg.nktc], f32, isOutput=False)
    if cfg.nmask:
        cmask = dp("cmask", [128, cfg.nmask, QT], bf16, isOutput=False)
    else:
        cmask = None
    out = dp("out", [NQT, NJT, 128, QT], bf16, isOutput=True)

    with tile.TileContext(nc) as tc:
        _body(nc, tc, cfg, locals())
    # populate .instr bytes for extended-inst ISA ops (custom DVE reciprocal);
    # Bacc.compile runs this pass but the raw-Bass path does not.
    from concourse.library_overlay import lower_extended_insts

    lower_extended_insts(nc)
    return nc


def _body(nc, tc, cfg, t):
    xS, xcA, xcB, wqf, wkf, wvf, wof = (
        t["xS"], t["xcA"], t["xcB"], t["wqf"], t["wkf"], t["wvf"], t["wof"]
    )
    bq, bk, bv, padb, cmask, out = (
        t["bq"], t["bk"], t["bv"], t["padb"], t["cmask"], t["out"]
    )
    SC, NKTC, NCT = cfg.sc, cfg.nktc, cfg.nct

    ctx_pools = []

    def pool(name, bufs, space="SBUF"):
        p = tc.tile_pool(name=name, bufs=bufs, space=space)
        ctx_pools.append(p)
        return p.__enter__()

    const = pool("const", 1)
    probs_pool = pool("probs", 4)
    stage_pool = pool("stage", 2)
    heads_pool = pool("heads", 4)
    outsb_pool = pool("outsb", 3)

    ps_proj = pool("ps_proj", 2, space="PSUM")
    ps_st = pool("ps_st", 2, space="PSUM")
    ps_ot = pool("ps_ot", 2, space="PSUM")

    ones_f = const.tile([128, 128], f32)
    nc.any.memset(ones_f[:], 1.0)
    ones_b = const.tile([1, 128], bf16)
    with nc.allow_low_precision(reason="exact small ints in bf16"):
        nc.vector.tensor_copy(ones_b[0:1, :], ones_f[0:1, :])

    # HAM warmup: the PE would otherwise idle ~15us waiting for the input
    # DMAs and the first ~3.4us of real matmuls would run at the 1.2 GHz
    # cold clock. Burn the wait on dummy matmuls (results never read) so the
    # activity monitor reaches 8/8 before the first projection.
    zdum = const.tile([128, QT], bf16)
    nc.vector.memset(zdum[:], 0.0)
    for i in range(20):
        psd = ps_st.tile([128, QT], f32, tag="st", name=f"psd{i}")
        nc.tensor.matmul(psd[:], zdum[:, 0:128], zdum[:], start=True, stop=True)
    # preload the ACT exp table during the input-DMA wait — otherwise the
    # 1.3us ACT_TABLE_LOAD serializes in front of the first real exp
    warm_e = const.tile([1, 2], f32)
    nc.scalar.activation(
        warm_e[:], ones_f[0:1, 0:2], mybir.ActivationFunctionType.Exp,
        scale=1.0,
    )

    # ---- small consts first (tiny DMAs) ---------------------------------
    bq_t = const.tile([128, 2], f32)
    nc.sync.dma_start(bq_t[:], bq[:])
    bk_t = const.tile([128, 2], f32)
    nc.sync.dma_start(bk_t[:], bk[:])
    padb_t = const.tile([128, NKTC], f32)
    nc.sync.dma_start(padb_t[:], padb[:])
    bv_row = const.tile([1, HC], bf16)
    nc.sync.dma_start(bv_row[:], bv[:])

    # ---- critical-path inputs in dependency order ------------------------
    xt = const.tile([128, NQT, KCH, QT], bf16)   # full x, s-slice-major
    xc_t = const.tile([128, KCH, SC], bf16)      # compacted-key x
    wq_t = const.tile([128, 2, KCH, 128], bf16)
    wk_t = const.tile([128, 2, KCH, 128], bf16)
    wv_t = const.tile([128, KCH, HC], bf16)
    wo_t = const.tile([128, 2, NJT, 128], bf16)
    if cfg.nmask:
        cm_t = const.tile([128, cfg.nmask, QT], bf16)
    else:
        cm_t = None

    # First wave, chunked so the k-projection's inputs own every DMA ring:
    # wk jt0/jt1 (k-proj both jt) + xcA in 128-col slices.
    CA = min(SC, 512)
    for k4 in range(0, KCH, 4):
        nc.sync.dma_start(wk_t[:, 0, k4 : k4 + 4], wkf[:, 0, k4 : k4 + 4])
    for k4 in range(0, KCH, 4):
        nc.sync.dma_start(wk_t[:, 1, k4 : k4 + 4], wkf[:, 1, k4 : k4 + 4])
    for c4 in range(0, CA, 128):
        nc.sync.dma_start(
            xc_t[:, :, c4 : c4 + 128], xcA[:, :, c4 : c4 + 128]
        )
    # Second wave: q-projection inputs. Gated on wave-1 completion (the
    # dummy copy reads the last xcA chunk and writes a wq cell, and the SP
    # queue dispatches DMA triggers in order) so the k-projection's inputs
    # get the full HBM bandwidth and the PE can start earlier.
    with nc.allow_low_precision(reason="dummy gate, overwritten"):
        nc.vector.tensor_copy(wq_t[:, 0, 0, 0:1], xc_t[:, KCH - 1, CA - 1 : CA])
    nc.sync.dma_start(wq_t[:, 0], wqf[:, 0])
    nc.sync.dma_start(wq_t[:, 1], wqf[:, 1])
    nc.sync.dma_start(xt[:, 0], xS[0])
    nc.sync.dma_start(wv_t[:], wvf[:])
    if cfg.nmask and cfg.nmask_q0:
        nc.sync.dma_start(cm_t[:, 0 : cfg.nmask_q0], cmask[:, 0 : cfg.nmask_q0])

    bvb = const.tile([128, HC], f32)
    bv_ps = ps_proj.tile([128, HC], f32, tag="proj")
    nc.tensor.matmul(bv_ps[:], ones_b[:], bv_row[:], start=True, stop=True)
    nc.vector.tensor_copy(bvb[:], bv_ps[:])

    # projection outputs
    qh_t = const.tile([128, 2, S], bf16)    # qhT: [j-in-tile, j-tile, s]
    kh_t = const.tile([128, 2, SC], bf16)
    vh_t = const.tile([128, NKTC, HPC, HD + 1], bf16)
    with nc.allow_low_precision(reason="exact small ints in bf16"):
        nc.vector.tensor_copy(
            vh_t[:, :, :, HD : HD + 1].rearrange("p t h o -> p (t h) o"),
            ones_f[:, 0 : NKTC * HPC].rearrange("p (f o) -> p f o", o=1),
        )

    def bulk_loads():
        """Deferred loads. The dummy copy makes the first bulk DMA depend on
        the last critical load (xS[0]); since the SP queue dispatches DMA
        triggers in program order, every bulk transfer queues behind it and
        cannot starve the critical-path loads of HBM bandwidth."""
        with nc.allow_low_precision(reason="dummy gate, overwritten"):
            nc.vector.tensor_copy(
                xt[:, 1, 0, 0:1], xt[:, 0, KCH - 1, QT - 1 : QT]
            )
        nc.sync.dma_start(xt[:, 1], xS[1])
        if xcB is not None:
            nc.sync.dma_start(xc_t[:, :, 512:SC], xcB[:])
        if cfg.nmask and cfg.nmask_q0 < cfg.nmask:
            nc.sync.dma_start(
                cm_t[:, cfg.nmask_q0 :], cmask[:, cfg.nmask_q0 :]
            )
        nc.sync.dma_start(wo_t[:], wof[:])
        nc.sync.dma_start(xt[:, 2], xS[2])
        nc.sync.dma_start(xt[:, 3], xS[3])

    def proj_q(jt, c):
        """one [128, 512] tile of qhT: out partition=j, free=s."""
        ps = ps_proj.tile([128, QT], f32, tag="proj")
        for k in range(KCH):
            nc.tensor.matmul(
                ps[:],
                wq_t[:, jt, k, :],
                xt[:, c, k, :],
                start=(k == 0),
                stop=(k == KCH - 1),
            )
            if k % 2 == 1:
                yield
        with nc.allow_low_precision(reason="bf16 activations"):
            nc.vector.tensor_scalar_add(
                qh_t[:, jt, c * QT : (c + 1) * QT], ps[:], bq_t[:, jt : jt + 1]
            )

    def proj_k(jt, ct):
        """one column tile of khT over compacted keys."""
        c0 = ct * 512
        w = min(512, SC - c0)
        ps = ps_proj.tile([128, QT], f32, tag="proj")
        for k in range(KCH):
            nc.tensor.matmul(
                ps[:, 0:w],
                wk_t[:, jt, k, :],
                xc_t[:, k, c0 : c0 + w],
                start=(k == 0),
                stop=(k == KCH - 1),
            )
            if k % 2 == 1:
                yield
        with nc.allow_low_precision(reason="bf16 activations"):
            nc.vector.tensor_scalar_add(
                kh_t[:, jt, c0 : c0 + w], ps[:, 0:w], bk_t[:, jt : jt + 1]
            )

    def proj_v(sb):
        """one compacted s-block of vh: out partition=s, free=[4 heads x 64]."""
        ps = ps_proj.tile([128, HC], f32, tag="proj")
        for k in range(KCH):
            nc.tensor.matmul(
                ps[:],
                xc_t[:, k, sb * 128 : (sb + 1) * 128],
                wv_t[:, k, :],
                start=(k == 0),
                stop=(k == KCH - 1),
            )
            if k % 2 == 1:
                yield
        with nc.allow_low_precision(reason="bf16 activations"):
            nc.vector.tensor_tensor(
                vh_t[:, sb, :, 0:HD],
                ps[:].rearrange("p (h d) -> p h d", h=HPC),
                bvb[:].rearrange("p (h d) -> p h d", h=HPC),
                mybir.AluOpType.add,
            )

    heads_sb = {}

    def stage_phase1(qi, pair, ot0, ot1):
        """ACT/DVE-only half of normalization: reciprocal of the denominator
        rows and PSUM->SBUF copies of the head outputs (frees the ot banks).
        No PE instructions, so the tensor engine never waits on this chain."""
        raw = stage_pool.tile([128, QT], f32, tag="raw", bufs=4)
        den_w = stage_pool.tile([1, 2, QT], f32, tag="denw")
        # drain the ot banks on BOTH ACT and DVE so neither queue blocks the
        # next pair's exps for long and the banks free as fast as possible
        nc.scalar.copy(den_w[0:1, 0], ot0[HD : HD + 1, :])
        nc.vector.tensor_copy(den_w[0:1, 1], ot1[HD : HD + 1, :])
        nc.scalar.copy(raw[0:HD, :], ot0[0:HD, :])
        nc.vector.tensor_copy(raw[HD : 2 * HD, :], ot1[0:HD, :])
        den_r = stage_pool.tile([1, 2, QT], f32, tag="den")
        den_b = stage_pool.tile([1, 2, QT], bf16, tag="denb")
        with nc.allow_low_precision(reason="approx reciprocal, bf16 denoms"):
            nc.vector.reciprocal_approx_fast(
                den_r[0:1, :, :].rearrange("o a q -> o (a q)"),
                den_w[0:1, :, :].rearrange("o a q -> o (a q)"),
            )
            nc.vector.tensor_copy(den_b[:], den_r[:])
        return raw, den_b

    def stage_phase2(qi, pair, raw, den_b):
        """PE broadcast of the reciprocal denominators (col-tiled pair)
        + normalize multiply into the bf16 heads tile the O-projection
        consumes. Deferred into the next pair's kt-loop."""
        bcst = ps_st.tile([128, QT], f32, tag="st")
        nc.tensor.matmul(
            bcst[0:HD, :], ones_b[0:1, 0:HD], den_b[0:1, 0, :],
            start=True, stop=True,
        )
        nc.tensor.matmul(
            bcst[HD:128, :], ones_b[0:1, 0:HD], den_b[0:1, 1, :],
            start=True, stop=True,
        )
        h = heads_pool.tile([128, QT], bf16, tag="heads", name=f"h{qi}_{pair}")
        with nc.allow_low_precision(reason="bf16 staging"):
            nc.vector.tensor_tensor(
                h[:], raw[:], bcst[:], mybir.AluOpType.mult
            )
        heads_sb[(qi, pair)] = h

    def make_finish(qi, pair, raw, den_r):
        def fin():
            stage_phase2(qi, pair, raw, den_r)
        return fin

    def attention_qtile(qi, finishq, filler=None, defer_last=False):
        q0 = qi * QT
        nk = cfg.nktq[qi]
        for pair in range(2):
            ot0 = ps_ot.tile([HD + 1, QT], f32, tag="ot")
            ot1 = ps_ot.tile([HD + 1, QT], f32, tag="ot")
            ots = (ot0, ot1)
            for kt in range(nk):
                if filler is not None:
                    filler()
                if kt == min(3, nk - 1) and finishq:
                    finishq.pop(0)()
                d0 = cfg.d0[qi][kt]
                st = ps_st.tile([128, 2, QT], f32, tag="st")
                for hh in range(2):
                    nc.tensor.matmul(
                        st[:, hh, d0:QT],
                        kh_t[hh * 64 : hh * 64 + 64, pair, kt * 128 : kt * 128 + 128],
                        qh_t[hh * 64 : hh * 64 + 64, pair, q0 + d0 : q0 + QT],
                        start=True,
                        stop=True,
                    )
                probs = probs_pool.tile([128, 2, QT], bf16, tag="probs")
                with nc.allow_low_precision(reason="bf16 probs"):
                    nc.scalar.activation(
                        probs[:, :, d0:QT],
                        st[:, :, d0:QT],
                        mybir.ActivationFunctionType.Exp,
                        bias=padb_t[:, kt : kt + 1],
                        scale=float(SCALE),
                    )
                mi = cfg.mask_idx.get((qi, kt))
                if mi is not None:
                    # causal boundary mask: bf16 0/1 multiply on the probs
                    # (host-precomputed; exp of an unmasked future score is
                    # at most ~e^8, no overflow before the zeroing)
                    with nc.allow_low_precision(reason="bf16 probs"):
                        nc.vector.tensor_tensor(
                            probs[:, :, d0:QT],
                            probs[:, :, d0:QT],
                            cm_t[:, mi, d0:QT].rearrange(
                                "p (o n) -> p o n", o=1
                            ).broadcast_to([128, 2, QT - d0]),
                            mybir.AluOpType.mult,
                        )
                for hh in range(2):
                    h = 2 * pair + hh
                    nc.tensor.matmul(
                        ots[hh][:, d0:QT],
                        vh_t[:, kt, h, :],
                        probs[:, hh, d0:QT],
                        start=(kt == 0),
                        stop=(kt == nk - 1),
                    )
            if pair == 1 and defer_last:
                return ot0, ot1
            raw, den_r = stage_phase1(qi, pair, ot0, ot1)
            finishq.append(make_finish(qi, pair, raw, den_r))

    def oproj_tile(c):
        """Full-width O-projection partial for q-tile c from this core's own
        normalized heads (row-parallel Wo; host sums the 4 partials)."""
        while finishq and ((c, 0) not in heads_sb or (c, 1) not in heads_sb):
            finishq.pop(0)()
        for jt in range(NJT):
            ps = ps_proj.tile([128, QT], f32, tag="proj")
            nc.tensor.matmul(
                ps[:], wo_t[:, 0, jt, :], heads_sb[(c, 0)][:],
                start=True, stop=False,
            )
            nc.tensor.matmul(
                ps[:], wo_t[:, 1, jt, :], heads_sb[(c, 1)][:],
                start=False, stop=True,
            )
            osb = outsb_pool.tile([128, QT], bf16, tag="osb")
            with nc.allow_low_precision(reason="bf16 output partials"):
                nc.vector.tensor_copy(osb[:], ps[:])
            nc.sync.dma_start(out[c, jt], osb[:])
            yield

    # ---- emission: projections + O-proj finely interleaved with attention -
    def units_for(stage):
        """stage 0: prereqs of attention(0); stage qi+1: work to interleave
        during attention(qi) (prereqs of qi+1, plus oproj(qi-1))."""
        u = []
        if stage == 0:
            for jt in range(2):
                u.append(lambda jt=jt: proj_k(jt, 0))
            for jt in range(2):
                u.append(lambda jt=jt: proj_q(jt, 0))
            for sb in range(cfg.nktq[0]):
                u.append(lambda sb=sb: proj_v(sb))
            return u
        qi = stage - 1  # currently-running attention tile
        if qi + 1 < NQT:
            if qi + 1 < NCT:  # k-proj tile qi+1 (cols beyond 512*(qi+1))
                for jt in range(2):
                    u.append(lambda jt=jt, ct=qi + 1: proj_k(jt, ct))
            for sb in range(cfg.nktq[qi], cfg.nktq[qi + 1]):
                u.append(lambda sb=sb: proj_v(sb))
            for jt in range(2):
                u.append(lambda jt=jt, c=qi + 1: proj_q(jt, c))
        if qi >= 1:
            u.append(lambda c=qi - 1: oproj_tile(c))
        return u

    class Filler:
        def __init__(self, units, budget, skip=0):
            self.units = list(units)
            self.gen = None
            self.budget = budget
            self.skip = skip

        def __call__(self):
            if self.skip > 0:
                self.skip -= 1
                return
            for _ in range(self.budget):
                if self.gen is None:
                    if not self.units:
                        return
                    self.gen = self.units.pop(0)()
                try:
                    next(self.gen)
                except StopIteration:
                    self.gen = None

        def flush(self):
            while self.units or self.gen is not None:
                if self.gen is None:
                    self.gen = self.units.pop(0)()
                for _ in self.gen:
                    pass
                self.gen = None

    # stage 0 prereqs, with bulk loads triggered after the first k-proj
    init_units = units_for(0)
    first = Filler([init_units[0]], 1)
    first.flush()
    bulk_loads()
    Filler(init_units[1:], 1).flush()

    finishq = []
    last_ots = None
    for qi in range(NQT):
        pending = units_for(qi + 1)
        n_att = 2 * cfg.nktq[qi]
        total_steps = len(pending) * 5
        # when the only pending work is an O-projection, hold it back until
        # the deferred phase2 of its second head pair has been popped
        skip = 4 if (pending and len(pending) == 1 and qi >= 1) else 0
        budget = max(1, -(-total_steps // max(1, n_att - skip)))
        filler = Filler(pending, budget, skip=skip)
        last_ots = attention_qtile(
            qi, finishq, filler, defer_last=(qi == NQT - 1)
        )
        filler.flush()

    # tail: tile-3 pair-1 normalization, then its O-projection. The first
    # head-pair's contribution for 4 column tiles is pre-started into spare
    # PSUM slots so the PE isn't idle during the reciprocal chain. (The st
    # pool must stay untouched here: phase2's bcst allocates from it.)
    raw, den_r = stage_phase1(NQT - 1, 1, *last_ots)
    while finishq:
        finishq.pop(0)()
    c3 = NQT - 1
    pre = []
    for jt in range(4):
        pool_ = ps_proj if jt < 2 else ps_ot
        tag_ = "proj" if jt < 2 else "ot"
        ps = pool_.tile([128, QT], f32, tag=tag_, name=f"otail{jt}")
        nc.tensor.matmul(
            ps[:], wo_t[:, 0, jt, :], heads_sb[(c3, 0)][:],
            start=True, stop=False,
        )
        pre.append(ps)
    stage_phase2(c3, 1, raw, den_r)
    for jt in range(NJT):
        if jt < 4:
            ps = pre[jt]
        else:
            ps = ps_proj.tile([128, QT], f32, tag="proj")
            nc.tensor.matmul(
                ps[:], wo_t[:, 0, jt, :], heads_sb[(c3, 0)][:],
                start=True, stop=False,
            )
        nc.tensor.matmul(
            ps[:], wo_t[:, 1, jt, :], heads_sb[(c3, 1)][:],
            start=False, stop=True,
        )
        # tail-only: ACT is idle here (no more exps), so alternate the
        # PSUM->SBUF casts across both engines — otherwise 8 serial DVE
        # casts pace the final 16 matmuls and the output drain.
        osb = outsb_pool.tile([128, QT], bf16, tag="osb")
        with nc.allow_low_precision(reason="bf16 output partials"):
            if jt % 2 == 0:
                nc.vector.tensor_copy(osb[:], ps[:])
            else:
                nc.scalar.copy(osb[:], ps[:])
        nc.sync.dma_start(out[c3, jt], osb[:])

    for p in reversed(ctx_pools):
        p.__exit__(None, None, None)


# ---- host-side marshalling ----------------------------------------------


def _bf16(a):
    import ml_dtypes

    return np.ascontiguousarray(
        np.asarray(a, dtype=np.float32).astype(ml_dtypes.bfloat16)
    )


def _wswizzle(WT):
    """[D, 256] -> [128, 2, KCH, 128]: w[p, jt, k, j] = WT[k*128+p, jt*128+j]."""
    return WT.reshape(KCH, 128, 2, 128).transpose(1, 2, 0, 3)


def make_inputs(q, pad_mask, Wq, bq, Wk, bk, Wv, bv, Wo, bo, cfg=None):
    """Build the 8 per-core input maps from full inputs."""
    if cfg is None:
        cfg = Cfg(pad_mask)
    SC, NKTC = cfg.sc, cfg.nktc
    in_maps = []
    xSs, xcAs, xcBs, padbs, cms = [], [], [], [], []
    for b in range(B):
        xb = np.asarray(q[b], dtype=np.float32)
        xSs.append(
            _bf16(xb.T.reshape(KCH, 128, NQT, QT).transpose(2, 1, 0, 3))
        )
        keys = cfg.keys[b]
        scb = cfg.scb[b]
        xcT = np.zeros((D, SC), np.float32)
        xcT[:, :scb] = xb[keys].T
        xc = xcT.reshape(KCH, 128, SC).transpose(1, 0, 2)
        xcAs.append(_bf16(xc[:, :, 0 : min(SC, 512)]))
        xcBs.append(_bf16(xc[:, :, 512:SC]) if SC > 512 else None)
        jj = np.arange(SC)
        padbs.append(
            np.ascontiguousarray(
                np.where(jj < scb, np.float32(0), np.float32(NEG))
                .reshape(NKTC, 128)
                .T
            )
        )
        if cfg.nmask:
            cm = np.zeros((128, cfg.nmask, QT), np.float32)
            qf = np.arange(QT)
            for i, (qi, kt) in enumerate(cfg.mask_order):
                j = kt * 128 + np.arange(128)
                valid = j < scb
                pos = keys[np.minimum(j, scb - 1)]
                cm[:, i, :] = np.where(
                    valid[:, None] & (pos[:, None] > qi * QT + qf[None, :]),
                    np.float32(0),
                    np.float32(1),
                )
            cms.append(_bf16(cm))
        else:
            cms.append(None)

    WoT = np.ascontiguousarray(np.asarray(Wo, dtype=np.float32).T)  # [d, j]
    for core in range(8):
        b, r = divmod(core, GROUP)
        sl = slice(r * HC, (r + 1) * HC)
        # wof rows: chunk pair holds rows hh*64+dd of heads 2*pair+hh
        rows = np.array(
            [
                (r * HPC + 2 * pair + hh) * HD + dd
                for pair in range(2)
                for hh in range(2)
                for dd in range(HD)
            ],
            dtype=np.int64,
        )
        wo4 = WoT[rows].reshape(2, 128, NJT, 128).transpose(1, 0, 2, 3)
        im = {
            "xS": xSs[b],
            "xcA": xcAs[b],
            "wqf": _bf16(_wswizzle(np.asarray(Wq, np.float32)[sl, :].T)),
            "wkf": _bf16(_wswizzle(np.asarray(Wk, np.float32)[sl, :].T)),
            "wvf": _bf16(
                np.asarray(Wv, np.float32)[sl, :].T
                .reshape(KCH, 128, HC).transpose(1, 0, 2)
            ),
            "wof": _bf16(wo4),
            "bq": np.ascontiguousarray(np.asarray(bq)[sl].reshape(2, 128).T),
            "bk": np.ascontiguousarray(np.asarray(bk)[sl].reshape(2, 128).T),
            "bv": _bf16(np.asarray(bv)[sl].reshape(1, HC)),
            "padb": padbs[b],
        }
        if SC > 512:
            im["xcB"] = xcBs[b]
        if cfg.nmask:
            im["cmask"] = cms[b]
        in_maps.append(im)
    return in_maps


def assemble_output(results, bo):
    full = np.zeros((B, S, D), dtype=np.float32)
    for core in range(8):
        b, _ = divmod(core, GROUP)
        o = np.asarray(results[core]["out"], dtype=np.float32)
        # out[c, jt, j, q] = partial for row c*512+q, col jt*128+j
        full[b] += o.transpose(0, 3, 1, 2).reshape(S, D)
    full += np.asarray(bo, dtype=np.float32)[None, None, :]
    return full


_NC_CACHE = [None, None]  # [cfg.key, nc]


def _get_nc(cfg):
    if _NC_CACHE[0] != cfg.key:
        _NC_CACHE[1] = build(cfg)
        _NC_CACHE[0] = cfg.key
    return _NC_CACHE[1]


def kernel(**inputs):
    """Full-input MHA forward. inputs: q, pad_mask, Wq, bq, Wk, bk, Wv, bv,
    Wo, bo (as produced by setup_inputs). Returns [B, S, D] float32."""
    inputs = {k: np.asarray(v) for k, v in inputs.items()}
    cfg = Cfg(inputs["pad_mask"])
    nc = _get_nc(cfg)
    in_maps = make_inputs(**inputs, cfg=cfg)
    res = run_bass_kernel_spmd(nc, in_maps, list(range(8)))
    return assemble_output(res.results, inputs["bo"])
